# revision 1
# baseline (speedup 1.0000x reference)
"""TRN2 Bass kernel for nn_BlockMoVaE (attention + MoE/VE routing block).

Self-contained: accepts FULL inputs, shards across 8 NeuronCores, returns
FULL output.

Sharding:
  Phase 1 (attention + router logits): token-parallel. Core c handles the
    512-query strip [qoff, qoff+512) of batch b=c//4, qoff=512*(c%4).
    Activations are kept FEATURE-major ([feature, token]) so no on-device
    transposes are needed. K/V are computed for the whole batch on each
    core of the batch group, with key 128-tiles stored in a per-core
    ROTATED slot order (slot s holds absolute key tile (qoff/128+s)%16) so
    the causal boundary lands at static slots 0..3 in every core's
    (shared, SPMD) program; fully-masked future tiles are killed by a
    per-slot additive bias (-3e4) inside the exp activation.
  Phase 2 (expert-parallel sparse MoE): core e computes MLP expert e over
    only the tokens routed to it (host gathers columns, capacity-padded);
    VE (vocab-embedding expert) rows are host-gathered and weighted on
    device per token strip. Host does top-2 routing between launches and
    the final scatter-add/assembly.

Matmuls run as float32r (full PE rate, ~1e-4 rel err); PSUM accumulates
in fp32.
"""
import numpy as np

import concourse.bass as bass
import concourse.bacc as bacc
import concourse.mybir as mybir
import concourse.tile as tile
from concourse.bass_utils import run_bass_kernel_spmd

# ---- problem constants (hardcoded per contest rules) ----
B, T, C = 2, 2048, 1024
NH, NKV, HD = 16, 8, 64
E_MLP, E_VE, TOPK = 8, 2, 2
HID = 2048
VOCAB = 50257
EPS = 1e-6
NCORES = 8
S = 512            # tokens per core strip
NSLOT = T // 128   # 16 key tiles per batch
NG = 4             # kv column groups of 512
NCAP = 1024        # expert token capacity (phase 2)

f32 = mybir.dt.float32
f32r = mybir.dt.float32r
bf16 = mybir.dt.bfloat16
AF = mybir.ActivationFunctionType

_prog_cache = {}



def _register_consts(nc, values):
    for value in values:
        key = (f32, float(value))
        if key not in nc.const_aps.aps:
            t = nc.alloc_sbuf_tensor(f"constap-{value}", [128, 1], f32)
            nc.gpsimd.memset(t.ap(), float(value))
            nc.const_aps.aps[key] = t.ap()
    nc.all_engine_barrier()


# --------------------------------------------------------------------------
# Phase 1 builder: attention + residual + rmsnorm + router logits
# --------------------------------------------------------------------------
def build_phase1(window: int):
    nc = bacc.Bacc("TRN2", target_bir_lowering=False, debug=False,
                   num_devices=NCORES)

    xT_b = nc.dram_tensor("xT_b", [C, T], f32r, kind="ExternalInput").ap()
    xT_s = xT_b[:, 0:S]          # strip == rotated slots 0..3
    cosR_b = nc.dram_tensor("cosR_b", [128, T], f32, kind="ExternalInput").ap()
    sinR_b = nc.dram_tensor("sinR_b", [128, T], f32, kind="ExternalInput").ap()
    cosR_s = cosR_b[:, 0:S]
    sinR_s = sinR_b[:, 0:S]
    kbias = nc.dram_tensor("kbias", [128, NSLOT], f32, kind="ExternalInput").ap()
    wqT = nc.dram_tensor("wqT", [C, NH * HD], f32r, kind="ExternalInput").ap()
    wkT = nc.dram_tensor("wkT", [C, NKV * HD], f32r, kind="ExternalInput").ap()
    wvT = nc.dram_tensor("wvT", [C, NKV * HD], f32r, kind="ExternalInput").ap()
    woT = nc.dram_tensor("woT", [C, C], f32r, kind="ExternalInput").ap()
    rwT = nc.dram_tensor("rwT", [C, E_MLP + E_VE], f32, kind="ExternalInput").ap()

    x2_out = nc.dram_tensor("x2_out", [C, S], f32, kind="ExternalOutput").ap()
    xf_out = nc.dram_tensor("xf_out", [C, S], f32r, kind="ExternalOutput").ap()
    logit_out = nc.dram_tensor("logit_out", [E_MLP + E_VE, S], f32,
                               kind="ExternalOutput").ap()

    _register_consts(nc, [EPS])
    from contextlib import ExitStack
    with tile.TileContext(nc) as tc, ExitStack() as est:
        const = est.enter_context(tc.tile_pool(name="const", bufs=1))
        ropes = est.enter_context(tc.tile_pool(name="ropes", bufs=1))
        ropeb = est.enter_context(tc.tile_pool(name="ropeb", bufs=1))
        wstream = est.enter_context(tc.tile_pool(name="wstream", bufs=2))
        wvp = est.enter_context(tc.tile_pool(name="wvp", bufs=1))
        xin = est.enter_context(tc.tile_pool(name="xin", bufs=1))
        kvp = est.enter_context(tc.tile_pool(name="kv", bufs=1))
        qp = est.enter_context(tc.tile_pool(name="qp", bufs=1))
        work = est.enter_context(tc.tile_pool(name="work", bufs=2))
        rows = est.enter_context(tc.tile_pool(name="rows", bufs=1))
        pexp = est.enter_context(tc.tile_pool(name="pexp", bufs=3))
        ypool = est.enter_context(tc.tile_pool(name="ypool", bufs=1))
        x2p = est.enter_context(tc.tile_pool(name="x2p", bufs=1))
        ps_acc = est.enter_context(tc.tile_pool(name="ps_acc", bufs=2, space="PSUM"))
        ps_row = est.enter_context(tc.tile_pool(name="ps_row", bufs=1, space="PSUM"))
        ps_bc = est.enter_context(tc.tile_pool(name="ps_bc", bufs=1, space="PSUM"))
        ps_att = est.enter_context(tc.tile_pool(name="ps_att", bufs=2, space="PSUM"))

        # ---- constants ----
        ones_col_f = const.tile([128, 1], f32, name="ones_col_f")
        nc.vector.memset(ones_col_f[:], 1.0)
        ones_col = const.tile([128, 1], f32r, name="ones_col")
        nc.scalar.copy(ones_col[:], ones_col_f[:])
        ones_row_f = const.tile([1, 128], f32, name="ones_row_f")
        nc.vector.memset(ones_row_f[:], 1.0)
        ones_row = const.tile([1, 128], f32r, name="ones_row")
        nc.scalar.copy(ones_row[:], ones_row_f[:])
        onescols = const.tile([128, NKV, 1], f32, name="onescols")
        nc.vector.memset(onescols[:], 1.0)
        onescols_r = const.tile([128, NKV, 1], f32r, name="onescols_r")
        nc.vector.tensor_copy(onescols_r[:], onescols[:])
        kb = const.tile([128, NSLOT], f32, name="kb")
        nc.sync.dma_start(kb[:], kbias[:])

        cs = ropes.tile([128, S], f32, name="cs")
        nc.sync.dma_start(cs[:], cosR_s[:])
        ss = ropes.tile([128, S], f32, name="ss")
        nc.sync.dma_start(ss[:], sinR_s[:])

        rw_t = [const.tile([128, E_MLP + E_VE], f32, tag=f"rw{i}",
                           name=f"rw{i}") for i in range(8)]
        wv_t = [wvp.tile([128, NKV * HD], f32r, tag=f"wv{i}", name=f"wv{i}")
                for i in range(8)]
        for i in range(8):
            nc.sync.dma_start(rw_t[i][:], rwT[bass.ts(i, 128), :])
            nc.sync.dma_start(wv_t[i][:], wvT[bass.ts(i, 128), :])

        # ---- helper: rms broadcast for feature-major tiles ----
        def rms_stats(xtiles, n, nfeat):
            ssq = ps_row.tile([1, n], f32, tag="row", name="ssq")
            for i, xt in enumerate(xtiles):
                sq = work.tile([128, n], f32r, tag="sqstat", name="sqstat", bufs=1)
                nc.vector.tensor_mul(sq[:], xt[:], xt[:])
                nc.tensor.matmul(ssq[:], ones_col[:], sq[:],
                                 start=(i == 0), stop=(i == len(xtiles) - 1))
            srow = rows.tile([1, n], f32, tag="srow", name="srow")
            nc.scalar.activation(srow[:], ssq[:], AF.Sqrt,
                                 bias=EPS, scale=1.0 / nfeat)
            rrow = rows.tile([1, n], f32r, tag="rrow", name="rrow")
            with nc.allow_low_precision(reason="f32r rms bcast rows"):
                nc.vector.reciprocal(rrow[:], srow[:])
            bc = ps_bc.tile([128, n], f32, tag="bc", name="bc")
            nc.tensor.matmul(bc[:], ones_row[:], rrow[:], start=True, stop=True)
            bcs = work.tile([128, n], f32, tag="bcstat", name="bcstat", bufs=1)
            nc.scalar.copy(bcs[:], bc[:])
            return bcs

        # ---- helper: rope + per-head rmsnorm on a projection psum ----
        def rope_norm(ps, cos_ap, sin_ap, n, out_tile, col0):
            # ps: [128, 2n] pair psum: cols 0:n = projection, n:2n = the same
            # projection with 32-row blocks swapped (computed by a second
            # matmul group with a column-swapped lhsT AP)
            swp = work.tile([128, n], f32, tag="swp", name="swp")
            nc.vector.tensor_mul(swp[:], ps[:, n:2 * n], sin_ap)
            t1 = work.tile([128, n], f32, tag="ropet1", name="ropet1")
            nc.vector.tensor_mul(t1[:], ps[:, 0:n], cos_ap)
            nc.vector.tensor_add(swp[:], t1[:], swp[:])   # roped value
            sq = work.tile([128, n], f32r, tag="ropet1", name="ropesq")
            nc.vector.tensor_mul(sq[:], swp[:], swp[:])
            for hh in range(2):
                p0 = 64 * hh
                ssqh = ps_row.tile([1, n], f32, tag="row", name="ssqh")
                nc.tensor.matmul(ssqh[:], ones_col[p0:p0 + 64, :],
                                 sq[p0:p0 + 64, :], start=True, stop=True)
                srow = rows.tile([1, n], f32, tag="srow", name="hsrow")
                nc.scalar.activation(srow[:], ssqh[:], AF.Sqrt,
                                     bias=EPS, scale=1.0 / HD)
                rrow = rows.tile([1, n], f32r, tag="rrow", name="hrrow")
                with nc.allow_low_precision(reason="f32r rms bcast rows"):
                    nc.vector.reciprocal(rrow[:], srow[:])
                bch = ps_bc.tile([64, n], f32, tag="bc", name="bch")
                nc.tensor.matmul(bch[:], ones_row[:, :64], rrow[:],
                                 start=True, stop=True)
                bcs = work.tile([128, n], f32, tag="hbc", name="hbc")
                nc.scalar.copy(bcs[p0:p0 + 64, :], bch[:])
                nc.vector.tensor_mul(
                    out_tile[p0:p0 + 64, col0:col0 + n],
                    swp[p0:p0 + 64, :], bcs[p0:p0 + 64, :])

        # ================= strip pipeline (Q) =================
        xs_t = [xin.tile([128, S], f32r, tag=f"xi{i}", name=f"xs{i}")
                for i in range(8)]
        for i in range(8):
            nc.sync.dma_start(xs_t[i][:], xT_s[bass.ts(i, 128), :])
        bc_s = rms_stats([t[:].bitcast(f32) for t in xs_t], S, C)
        xn_s = []
        for i in range(8):
            xr = xs_t[i][:]
            nc.vector.tensor_mul(xr, xr.bitcast(f32), bc_s[:])  # in-place norm
            xn_s.append(xr)

        qT = [qp.tile([128, S], f32r, tag=f"qT{i}", name=f"qT{i}")
              for i in range(8)]
        for dq in range(8):
            q_ps = ps_acc.tile([128, 2 * S], f32, tag="acc", name="q_ps")
            wsl = wstream.tile([128, C], f32r, tag="wq", name="wq_sl", bufs=1)
            nc.sync.dma_start(
                wsl[:].rearrange("p (a m) -> p a m", m=128),
                wqT[:, bass.ts(dq, 128)].rearrange("(a p) m -> p a m", p=128))
            wsw = wstream.tile([128, C], f32r, tag="wqsw", name="wq_sw",
                               bufs=1)
            nc.scalar.copy(
                wsw[:],
                wsl[:].rearrange("p (a h q c) -> p a h q c",
                                 h=2, q=2, c=32)[:, :, :, ::-1, :])
            for ci in range(8):
                nc.tensor.matmul(q_ps[:, 0:S], wsl[:, bass.ts(ci, 128)],
                                 xn_s[ci], start=(ci == 0), stop=(ci == 7))
            for ci in range(8):
                nc.tensor.matmul(q_ps[:, S:2 * S], wsw[:, bass.ts(ci, 128)],
                                 xn_s[ci], start=(ci == 0), stop=(ci == 7))
            rope_norm(q_ps, cs[:], ss[:], S, qT[dq], 0)

        # ================= batch pipeline (K, V) =================
        kT = [kvp.tile([128, T], f32r, tag=f"kT{i}", name=f"kT{i}")
              for i in range(4)]
        vaug = [kvp.tile([128, NKV * (HD + 1)], f32r, tag=f"va{i}",
                         name=f"va{i}") for i in range(NSLOT)]
        for g in range(NG):
            xb_t = [xin.tile([128, S], f32r, tag=f"xi{i}", name=f"xb{i}")
                    for i in range(8)]
            for i in range(8):
                nc.sync.dma_start(xb_t[i][:], xT_b[bass.ts(i, 128),
                                                   bass.ts(g, S)])
            cbg = ropeb.tile([128, S], f32, tag="cbg", name="cbg")
            nc.sync.dma_start(cbg[:], cosR_b[:, bass.ts(g, S)])
            sbg = ropeb.tile([128, S], f32, tag="sbg", name="sbg")
            nc.sync.dma_start(sbg[:], sinR_b[:, bass.ts(g, S)])
            bc_b = rms_stats([t[:].bitcast(f32) for t in xb_t], S, C)
            xn_b = []
            for i in range(8):
                xr = xb_t[i][:]
                nc.vector.tensor_mul(xr, xr.bitcast(f32), bc_b[:])
                xn_b.append(xr)
            for dk in range(4):
                k_ps = ps_acc.tile([128, 2 * S], f32, tag="acc", name="k_ps")
                wsl = wstream.tile([128, C], f32r, tag="wk", name="wk_sl",
                                   bufs=2)
                nc.sync.dma_start(
                    wsl[:].rearrange("p (a m) -> p a m", m=128),
                    wkT[:, bass.ts(dk, 128)].rearrange("(a p) m -> p a m",
                                                       p=128))
                wsw = wstream.tile([128, C], f32r, tag="wksw",
                                   name="wk_sw", bufs=1)
                nc.scalar.copy(
                    wsw[:],
                    wsl[:].rearrange("p (a h q c) -> p a h q c",
                                     h=2, q=2, c=32)[:, :, :, ::-1, :])
                for ci in range(8):
                    nc.tensor.matmul(k_ps[:, 0:S], wsl[:, bass.ts(ci, 128)],
                                     xn_b[ci], start=(ci == 0), stop=(ci == 7))
                for ci in range(8):
                    nc.tensor.matmul(k_ps[:, S:2 * S],
                                     wsw[:, bass.ts(ci, 128)],
                                     xn_b[ci], start=(ci == 0), stop=(ci == 7))
                rope_norm(k_ps, cbg[:], sbg[:], S, kT[dk], g * S)
            for tt in range(4):
                slot = g * 4 + tt
                v_ps = ps_acc.tile([128, NKV * HD], f32, tag="acc", name="v_ps")
                for ci in range(8):
                    nc.tensor.matmul(v_ps[:],
                                     xn_b[ci][:, bass.ts(tt, 128)],
                                     wv_t[ci][:], start=(ci == 0), stop=(ci == 7))
                va = vaug[slot]
                va3 = va[:].rearrange("p (h d) -> p h d", d=HD + 1)
                vp3 = v_ps[:].rearrange("p (h d) -> p h d", d=HD)
                nc.scalar.copy(va3[:, :, 0:HD], vp3[:, :, :])
                nc.vector.tensor_copy(va3[:, :, HD:HD + 1], onescols_r[:])

        # ================= attention =================
        yT = [ypool.tile([128, S], f32r, tag=f"yT{i}", name=f"yT{i}")
              for i in range(8)]
        for h in range(NH):
            kh = h // 2                       # kv head
            dk, kp0 = kh // 2, 64 * (kh % 2)  # kT chunk/partition offset
            # q head layout is host-permuted so its partition base matches
            # the kv head base (matmul requires equal bases)
            dq, qp0 = 2 * (h // 4) + (h % 2), 64 * ((h // 2) % 2)
            assert qp0 == kp0
            yv = ps_att.tile([HD + 1, S], f32, tag="yv", name="yv", bufs=2)
            for sp in range(NSLOT // 2):
                # two slots share one 2-bank psum tile and one exp op; the
                # per-slot dead bias is pair-uniform (dead range is slots
                # 4..15-qoff/128, always whole pairs)
                s2 = ps_acc.tile([128, 2 * S], f32, tag="acc", name="s2")
                for half in range(2):
                    s = 2 * sp + half
                    nc.tensor.matmul(
                        s2[:, half * S:(half + 1) * S],
                        kT[dk][kp0:kp0 + 64, bass.ts(s, 128)],
                        qT[dq][qp0:qp0 + 64, :], start=True, stop=True)
                pT = pexp.tile([128, 2 * S], f32r, tag="pT", name="pT")
                nc.scalar.activation(pT[:], s2[:], AF.Exp,
                                     bias=kb[:, 2 * sp:2 * sp + 1], scale=0.125)
                for half in range(2):
                    s = 2 * sp + half
                    pTh = pT[:, half * S:(half + 1) * S]
                    if s < 4:
                        nc.gpsimd.affine_select(
                            pTh, pTh, pattern=[[1, S]], base=-128 * s,
                            channel_multiplier=-1,
                            compare_op=mybir.AluOpType.is_ge, fill=0.0)
                        if window < 512 - 128 * s:
                            nc.gpsimd.affine_select(
                                pTh, pTh, pattern=[[1, S]],
                                base=-128 * s - window, channel_multiplier=-1,
                                compare_op=mybir.AluOpType.is_le, fill=0.0)
                    else:
                        # cover partially AND fully window-cut past slots:
                        # a fully-cut slot may be pair-unmasked (kbias is
                        # pair-granular), so affine-zero it here
                        m = NSLOT - s
                        if window < 128 * m + 511:
                            nc.gpsimd.affine_select(
                                pTh, pTh, pattern=[[1, S]],
                                base=128 * m - window, channel_multiplier=-1,
                                compare_op=mybir.AluOpType.is_le, fill=0.0)
                    nc.tensor.matmul(yv[:], vaug[s][:, 65 * kh:65 * kh + 65],
                                     pTh, start=(s == 0), stop=(s == NSLOT - 1))
            ry = rows.tile([1, S], f32r, tag="ry", name="ry", bufs=1)
            with nc.allow_low_precision(reason="f32r softmax denom row"):
                nc.vector.reciprocal(ry[:], yv[HD:HD + 1, :])
            bc_y = ps_bc.tile([64, S], f32, tag="bc", name="bc_y")
            nc.tensor.matmul(bc_y[:], ones_row[:, :64], ry[:],
                             start=True, stop=True)
            bcy_s = work.tile([128, S], f32, tag="hbc", name="bcy")
            nc.vector.tensor_copy(bcy_s[qp0:qp0 + 64, :], bc_y[:])
            nc.vector.tensor_mul(yT[dq][qp0:qp0 + 64, :], yv[0:HD, :],
                                 bcy_s[qp0:qp0 + 64, :])

        # ================= wo + residual + xf + router =================
        x2w = []
        for co in range(8):
            # ps_row is idle during attention, so wo accumulation can
            # overlap the attention tail instead of queueing on "acc" slots
            at_ps = ps_row.tile([128, S], f32, tag="row", name="at_ps")
            wsl = wstream.tile([128, C], f32r, tag="wo", name="wo_sl", bufs=2)
            nc.sync.dma_start(
                wsl[:].rearrange("p (a m) -> p a m", m=128),
                woT[:, bass.ts(co, 128)].rearrange("(a p) m -> p a m", p=128))
            for ci in range(8):
                nc.tensor.matmul(at_ps[:], wsl[:, bass.ts(ci, 128)],
                                 yT[ci][:], start=(ci == 0), stop=(ci == 7))
            xs2 = xin.tile([128, S], f32r, tag=f"xi{co}", name=f"xs2_{co}")
            nc.sync.dma_start(xs2[:], xT_s[bass.ts(co, 128), :])
            x2 = x2p.tile([128, S], f32, tag="x2w", name="x2w")
            nc.vector.tensor_add(x2[:], at_ps[:], xs2[:].bitcast(f32))
            nc.sync.dma_start(x2_out[bass.ts(co, 128), :], x2[:])
            # xf stats accumulate inline while x2 is still in SBUF (avoids
            # waiting on the DRAM round trip for the stats pass)
            sqf = work.tile([128, S], f32r, tag="sqstat", name="sqf", bufs=1)
            nc.vector.tensor_mul(sqf[:], x2[:], x2[:])
            if co == 0:
                ssq_f = ps_bc.tile([1, S], f32, tag="bc", name="ssq_f")
            nc.tensor.matmul(ssq_f[:], ones_col[:], sqf[:],
                             start=(co == 0), stop=(co == 7))
            x2w.append(x2)
        srow_f = rows.tile([1, S], f32, tag="srow", name="srow_f")
        nc.scalar.activation(srow_f[:], ssq_f[:], AF.Sqrt,
                             bias=EPS, scale=1.0 / C)
        rrow_f = rows.tile([1, S], f32r, tag="rrow", name="rrow_f")
        with nc.allow_low_precision(reason="f32r rms bcast rows"):
            nc.vector.reciprocal(rrow_f[:], srow_f[:])
        bcps_f = ps_bc.tile([128, S], f32, tag="bc", name="bcps_f")
        nc.tensor.matmul(bcps_f[:], ones_row[:], rrow_f[:],
                         start=True, stop=True)
        bc_f = work.tile([128, S], f32, tag="bcstat", name="bc_f", bufs=1)
        nc.scalar.copy(bc_f[:], bcps_f[:])
        # re-read x2 (streamed) only for the normalize apply
        x2r = [xin.tile([128, S], f32, tag=f"xi{i}", name=f"x2r{i}")
               for i in range(8)]
        for i in range(8):
            nc.sync.dma_start(x2r[i][:], x2_out[bass.ts(i, 128), :])
        rt_ps = ps_row.tile([E_MLP + E_VE, S], f32, tag="row", name="rt_ps")
        for i in range(8):
            xf = x2p.tile([128, S], f32r, tag="xf", name="xf")
            nc.vector.tensor_mul(xf[:], x2r[i][:], bc_f[:])
            nc.sync.dma_start(xf_out[bass.ts(i, 128), :], xf[:])
            nc.tensor.matmul(rt_ps[:], rw_t[i][:], xf[:].bitcast(f32),
                             start=(i == 0), stop=(i == 7))
        lg = rows.tile([E_MLP + E_VE, S], f32, tag="lg", name="lg", bufs=1)
        nc.scalar.copy(lg[:], rt_ps[:])
        nc.sync.dma_start(logit_out[:], lg[:])

    nc.compile()
    return nc


# --------------------------------------------------------------------------
# Phase 2 builder: sparse expert MLP + VE weighting
# --------------------------------------------------------------------------
def build_phase2(ncap: int):
    nc = bacc.Bacc("TRN2", target_bir_lowering=False, debug=False,
                   num_devices=NCORES)
    NT = ncap // 256

    xfg = nc.dram_tensor("xfg", [C, ncap], f32r, kind="ExternalInput").ap()
    w_upT = nc.dram_tensor("w_upT", [C, HID], f32r, kind="ExternalInput").ap()
    w_downT = nc.dram_tensor("w_downT", [HID, C], f32r,
                             kind="ExternalInput").ap()
    gate = nc.dram_tensor("gate", [1, ncap], f32r, kind="ExternalInput").ap()
    ve0 = nc.dram_tensor("ve0", [S, C], f32, kind="ExternalInput").ap()
    ve1 = nc.dram_tensor("ve1", [S, C], f32, kind="ExternalInput").ap()
    ve_g = nc.dram_tensor("ve_g", [128, 8], f32, kind="ExternalInput").ap()

    moe_out = nc.dram_tensor("moe_out", [C, ncap], f32, kind="ExternalOutput").ap()
    ve_out = nc.dram_tensor("ve_out", [S, C], f32, kind="ExternalOutput").ap()

    from contextlib import ExitStack
    with tile.TileContext(nc) as tc, ExitStack() as est:
        const = est.enter_context(tc.tile_pool(name="const", bufs=1))
        wpool = est.enter_context(tc.tile_pool(name="wpool", bufs=1))
        hpool = est.enter_context(tc.tile_pool(name="hpool", bufs=1))
        stream = est.enter_context(tc.tile_pool(name="stream", bufs=2))
        work = est.enter_context(tc.tile_pool(name="work", bufs=2))
        ps_h = est.enter_context(tc.tile_pool(name="ps_h", bufs=3, space="PSUM"))
        ps_o = est.enter_context(tc.tile_pool(name="ps_o", bufs=3, space="PSUM"))
        ps_b = est.enter_context(tc.tile_pool(name="ps_b", bufs=2, space="PSUM"))

        ones_row_f = const.tile([1, 128], f32)
        nc.vector.memset(ones_row_f[:], 1.0)
        ones_row = const.tile([1, 128], f32r)
        nc.scalar.copy(ones_row[:], ones_row_f[:])

        up_t = [wpool.tile([128, HID], f32r, tag=f"up{i}", name=f"up{i}") for i in range(8)]
        dn_t = [wpool.tile([128, C], f32r, tag=f"dn{i}", name=f"dn{i}") for i in range(16)]
        for i in range(8):
            nc.sync.dma_start(up_t[i][:], w_upT[bass.ts(i, 128), :])
        for i in range(16):
            nc.sync.dma_start(dn_t[i][:], w_downT[bass.ts(i, 128), :])
        veg = const.tile([128, 8], f32)
        nc.sync.dma_start(veg[:], ve_g[:])
        gate_sb = const.tile([1, ncap], f32r)
        nc.sync.dma_start(gate_sb[:], gate[:])

        for nt in range(NT):
            csl = bass.ts(nt, 256)
            xf_t = [stream.tile([128, 256], f32r, tag=f"xf{i}", name=f"xf{i}")
                    for i in range(8)]
            for i in range(8):
                nc.sync.dma_start(xf_t[i][:], xfg[bass.ts(i, 128), csl])
            g_ps = ps_b.tile([128, 256], f32)
            nc.tensor.matmul(g_ps[:], ones_row[:], gate_sb[:, csl],
                             start=True, stop=True)
            g_bc = work.tile([128, 256], f32, tag="gbc", name="gbc")
            nc.scalar.copy(g_bc[:], g_ps[:])
            hT = [hpool.tile([128, 256], f32r, tag=f"hT{i}", name=f"hT{i}")
                  for i in range(16)]
            for hc in range(16):
                h_ps = ps_h.tile([128, 256], f32)
                for ci in range(8):
                    nc.tensor.matmul(h_ps[:], up_t[ci][:, bass.ts(hc, 128)],
                                     xf_t[ci][:], start=(ci == 0),
                                     stop=(ci == 7))
                hr = work.tile([128, 256], f32, tag="hrelu", name="hrelu")
                nc.scalar.activation(hr[:], h_ps[:], AF.Relu)
                nc.vector.tensor_mul(hT[hc][:], hr[:], hr[:])
            for co in range(8):
                o_ps = ps_o.tile([128, 256], f32)
                for hc in range(16):
                    nc.tensor.matmul(o_ps[:], dn_t[hc][:, bass.ts(co, 128)],
                                     hT[hc][:], start=(hc == 0),
                                     stop=(hc == 15))
                ot = work.tile([128, 256], f32, tag="ot", name="ot")
                nc.vector.tensor_mul(ot[:], o_ps[:], g_bc[:])
                nc.sync.dma_start(moe_out[bass.ts(co, 128), csl], ot[:])

        # VE weighting for own token strip (token-major)
        for tt in range(4):
            rsl = bass.ts(tt, 128)
            r0 = stream.tile([128, C], f32, tag="ver0", name="ver0")
            r1 = stream.tile([128, C], f32, tag="ver1", name="ver1")
            nc.sync.dma_start(r0[:], ve0[rsl, :])
            nc.sync.dma_start(r1[:], ve1[rsl, :])
            nc.vector.tensor_scalar_mul(r0[:], r0[:], veg[:, 2 * tt:2 * tt + 1])
            nc.vector.tensor_scalar_mul(r1[:], r1[:],
                                        veg[:, 2 * tt + 1:2 * tt + 2])
            nc.vector.tensor_add(r0[:], r0[:], r1[:])
            nc.sync.dma_start(ve_out[rsl, :], r0[:])

    nc.compile()
    return nc


# --------------------------------------------------------------------------
# Host orchestration
# --------------------------------------------------------------------------
def _phase1_inputs(x, cos, sin, window, wq, wk, wv, wo, router_w):
    """Build per-core in_maps for phase 1."""
    cosT = np.ascontiguousarray(cos[0, :, 0, :].T)  # (32, T)
    sinT = np.ascontiguousarray(sin[0, :, 0, :].T)
    cosR = np.tile(cosT, (4, 1)).astype(np.float32)          # (128, T)
    sinR = np.tile(np.vstack([sinT, -sinT]), (2, 1)).astype(np.float32)

    # q-head placement permutation (see attention loop): head h lives at
    # chunk 2*(h//4)+(h%2), partition base 64*((h//2)%2)
    colmap = np.zeros(NH * HD, np.int64)
    for h in range(NH):
        pos = (2 * (h // 4) + (h % 2)) * 128 + 64 * ((h // 2) % 2)
        colmap[pos:pos + HD] = np.arange(h * HD, (h + 1) * HD)
    wqT = np.ascontiguousarray(wq.T[:, colmap])
    wkT = np.ascontiguousarray(wk.T)
    wvT = np.ascontiguousarray(wv.T)
    woT = np.ascontiguousarray(wo.T[colmap, :])
    rwT = np.ascontiguousarray(router_w.T)

    in_maps = []
    perms = []
    for c in range(NCORES):
        b, qi = c // 4, c % 4
        qoff = S * qi
        q128 = qoff // 128
        perm = [(q128 + s) % NSLOT for s in range(NSLOT)]
        perms.append(perm)
        xT = x[b].T  # (C, T)
        xT_rot = np.ascontiguousarray(
            xT.reshape(C, NSLOT, 128)[:, perm, :].reshape(C, T))
        cosR_b = np.ascontiguousarray(
            cosR.reshape(128, NSLOT, 128)[:, perm, :].reshape(128, T))
        sinR_b = np.ascontiguousarray(
            sinR.reshape(128, NSLOT, 128)[:, perm, :].reshape(128, T))
        # per-slot alive bias
        kbias = np.zeros((128, NSLOT), np.float32)
        alive_s = np.zeros(NSLOT, bool)
        for s in range(NSLOT):
            kt = perm[s]
            # any (q in [qoff, qoff+511], k in [kt*128, kt*128+127]) with
            # k <= q and q - k <= window?
            dmin = qoff - (kt * 128 + 127)
            dmax = qoff + S - 1 - kt * 128
            alive_s[s] = (dmax >= 0) and (dmin <= window)
        for sp in range(NSLOT // 2):
            # the device applies one bias per slot PAIR; window-cut dead
            # slots in a live pair are zeroed by the device affine instead
            if not (alive_s[2 * sp] or alive_s[2 * sp + 1]):
                kbias[:, 2 * sp:2 * sp + 2] = -30000.0
        in_maps.append(dict(
            xT_b=xT_rot,
            cosR_b=cosR_b, sinR_b=sinR_b, kbias=kbias,
            wqT=wqT, wkT=wkT, wvT=wvT, woT=woT, rwT=rwT,
        ))
    return in_maps, perms


def _route(logits, router_bias):
    """Top-2 routing exactly as the reference (on host, f32)."""
    sig = (1.0 / (1.0 + np.exp(-logits.astype(np.float32)))).astype(np.float32)
    sel = sig + router_bias[None, :].astype(np.float32)
    idx = np.argsort(-sel, axis=1, kind="stable")[:, :TOPK]
    tw = np.take_along_axis(sig, idx, axis=1)
    tw = tw / tw.sum(axis=1, keepdims=True)
    N = logits.shape[0]
    sparse_w = np.zeros((N, E_MLP + E_VE), np.float32)
    np.put_along_axis(sparse_w, idx, tw, axis=1)
    return sparse_w


def kernel(**inputs):
    x = np.asarray(inputs["x"], np.float32)
    token_ids = np.asarray(inputs["token_ids"])
    cos = np.asarray(inputs["cos"], np.float32)
    sin = np.asarray(inputs["sin"], np.float32)
    window = int(np.asarray(inputs["window_size"]))
    wq, wk, wv, wo = (np.asarray(inputs[k], np.float32)
                      for k in ("wq", "wk", "wv", "wo"))
    w_up = np.asarray(inputs["w_up"], np.float32)
    w_down = np.asarray(inputs["w_down"], np.float32)
    router_w = np.asarray(inputs["router_w"], np.float32)
    router_bias = np.asarray(inputs["router_bias"], np.float32)
    ve_tables = np.asarray(inputs["ve_tables"], np.float32)

    key1 = ("p1", window)
    if key1 not in _prog_cache:
        _prog_cache[key1] = build_phase1(window)
    nc1 = _prog_cache[key1]

    in_maps, _ = _phase1_inputs(x, cos, sin, window, wq, wk, wv, wo, router_w)
    res1 = run_bass_kernel_spmd(nc1, in_maps, list(range(NCORES))).results

    x2T = np.concatenate([res1[c]["x2_out"] for c in range(NCORES)], axis=1)
    xfT = np.concatenate([res1[c]["xf_out"] for c in range(NCORES)], axis=1)
    logits = np.concatenate([res1[c]["logit_out"].T for c in range(NCORES)],
                            axis=0)  # (N, 10)

    N = B * T
    sparse_w = _route(logits, router_bias)

    # dispatch
    ncap = NCAP
    idx_list, n_list = [], []
    for e in range(E_MLP):
        idx_e = np.nonzero(sparse_w[:, e])[0]
        idx_list.append(idx_e)
        n_list.append(len(idx_e))
    max_n = max(n_list)
    while ncap < max_n:
        ncap *= 2

    key2 = ("p2", ncap)
    if key2 not in _prog_cache:
        _prog_cache[key2] = build_phase2(ncap)
    nc2 = _prog_cache[key2]

    tok = token_ids.reshape(-1)
    in_maps2 = []
    for c in range(NCORES):
        e = c
        idx_e = idx_list[e]
        xfg = np.zeros((C, ncap), np.float32)
        xfg[:, :n_list[e]] = xfT[:, idx_e]
        gate = np.zeros((1, ncap), np.float32)
        gate[0, :n_list[e]] = sparse_w[idx_e, e]
        s0 = c * S
        strip_tok = tok[s0:s0 + S]
        ve0 = np.ascontiguousarray(ve_tables[0][strip_tok])
        ve1 = np.ascontiguousarray(ve_tables[1][strip_tok])
        veg = np.zeros((128, 8), np.float32)
        for tt in range(4):
            for ee in range(E_VE):
                veg[:, 2 * tt + ee] = sparse_w[s0 + tt * 128:s0 + (tt + 1) * 128,
                                               E_MLP + ee]
        in_maps2.append(dict(
            xfg=xfg, w_upT=np.ascontiguousarray(w_up[e].T),
            w_downT=np.ascontiguousarray(w_down[e].T),
            gate=gate, ve0=ve0, ve1=ve1, ve_g=veg,
        ))
    res2 = run_bass_kernel_spmd(nc2, in_maps2, list(range(NCORES))).results

    out = np.ascontiguousarray(x2T.T)  # (N, C)
    for c in range(NCORES):
        out[c * S:(c + 1) * S] += res2[c]["ve_out"]
    for e in range(E_MLP):
        n_e = n_list[e]
        if n_e:
            out[idx_list[e]] += res2[e]["moe_out"][:, :n_e].T
    return out.reshape(B, T, C).astype(np.float32)



# revision 2
# speedup vs baseline: 1.1175x; 1.1175x over previous
"""TRN2 Bass kernel v2 for nn_BlockMoVaE — head-sharded attention.

Sharding:
  Phase 1 (attention): core c = (b, g) with b = c//4, g = c%4 computes heads
    {4g..4g+3} (kv heads {2g, 2g+1}) of batch b over ALL T=2048 tokens, and
    outputs the wo-projected partial y contribution [C, T].  Each core sees
    the full causal triangle, so no slot is dead and total score/AV work is
    halved vs token-sharding.  Host reduces the 4 partials per batch, adds
    the residual, computes rmsnorm + router logits (tiny O(N*C) work) and
    does top-2 routing exactly as the reference.
  Phase 2 (expert MLP): core e runs expert e over its routed tokens
    (capacity-padded).  fp8(e4m3) weights/activations with DoubleRow
    matmuls; both weight matrices and the up-proj input are SPLIT into
    hi + lo fp8 pairs (3-pass up, 2-pass down) so the only surviving
    quantization error is the fp8 hidden state (~1e-2 of output max).
    VE (vocab-embedding) experts are a pure host-side gather + scale.

Key identity: rmsnorm(rope(rmsnorm(x) @ wq)) == rmsnorm(rope(x @ wq)) per
head, because the per-token rms scale commutes through the projection and
rope and cancels in the final per-head rmsnorm.  So Q/K are projected from
RAW x; only V needs the per-token 1/rms scale (applied on the psum copy).
"""
import numpy as np
import ml_dtypes

import concourse.bass as bass
import concourse.bacc as bacc
import concourse.mybir as mybir
import concourse.tile as tile
from concourse.bass_utils import run_bass_kernel_spmd

# ---- problem constants ----
B, T, C = 2, 2048, 1024
NH, NKV, HD = 16, 8, 64
E_MLP, E_VE, TOPK = 8, 2, 2
HID = 2048
VOCAB = 50257
EPS = 1e-6
NCORES = 8
NSLOT = T // 128      # 16
NG = 4                # token groups of 512 in phase 1
SG = T // NG          # 512
NCAP = 1024           # phase-2 expert capacity
WU = 64.0             # fp8 up-weight pre-scale
WD = 64.0             # fp8 down-weight pre-scale

f32 = mybir.dt.float32
f32r = mybir.dt.float32r
bf16 = mybir.dt.bfloat16
fp8 = mybir.dt.float8e4
E4NP = ml_dtypes.float8_e4m3
BF16NP = ml_dtypes.bfloat16
AF = mybir.ActivationFunctionType
DR = mybir.MatmulPerfMode.DoubleRow

_prog_cache = {}


def _register_consts(nc, values):
    for value in values:
        key = (f32, float(value))
        if key not in nc.const_aps.aps:
            t = nc.alloc_sbuf_tensor(f"constap-{value}", [128, 1], f32)
            nc.gpsimd.memset(t.ap(), float(value))
            nc.const_aps.aps[key] = t.ap()
    nc.all_engine_barrier()


# --------------------------------------------------------------------------
# Phase 1: head-sharded attention, partial wo output
# --------------------------------------------------------------------------
def build_phase1(window: int):
    nc = bacc.Bacc("TRN2", target_bir_lowering=False, debug=False,
                   num_devices=NCORES)

    xT = nc.dram_tensor("xT", [C, T], f32r, kind="ExternalInput").ap()
    cosR = nc.dram_tensor("cosR", [128, T], f32, kind="ExternalInput").ap()
    sinR = nc.dram_tensor("sinR", [128, T], f32, kind="ExternalInput").ap()
    wqT = nc.dram_tensor("wqT", [C, 256], f32r, kind="ExternalInput").ap()
    wkT = nc.dram_tensor("wkT", [C, 128], f32r, kind="ExternalInput").ap()
    wvT = nc.dram_tensor("wvT", [C, 128], f32r, kind="ExternalInput").ap()
    woT = nc.dram_tensor("woT", [256, C], f32r, kind="ExternalInput").ap()
    pswap = nc.dram_tensor("pswap", [128, 128], f32r, kind="ExternalInput").ap()
    pident = nc.dram_tensor("pident", [128, 128], f32r, kind="ExternalInput").ap()
    hmask = nc.dram_tensor("hmask", [128, 2], f32r, kind="ExternalInput").ap()
    hsel = nc.dram_tensor("hsel", [2, 128], f32r, kind="ExternalInput").ap()

    y_out = nc.dram_tensor("y_out", [C, T], f32, kind="ExternalOutput").ap()

    _register_consts(nc, [EPS, 0.0])
    from contextlib import ExitStack
    with tile.TileContext(nc) as tc, ExitStack() as est:
        const = est.enter_context(tc.tile_pool(name="const", bufs=1))
        wpool = est.enter_context(tc.tile_pool(name="wpool", bufs=1))
        ropes = est.enter_context(tc.tile_pool(name="ropes", bufs=1))
        xin = est.enter_context(tc.tile_pool(name="xin", bufs=2))
        qkv = est.enter_context(tc.tile_pool(name="qkv", bufs=1))
        vap = est.enter_context(tc.tile_pool(name="vap", bufs=1))
        work = est.enter_context(tc.tile_pool(name="work", bufs=2))
        rows = est.enter_context(tc.tile_pool(name="rows", bufs=2))
        pexp = est.enter_context(tc.tile_pool(name="pexp", bufs=4))
        ypool = est.enter_context(tc.tile_pool(name="ypool", bufs=2))
        obuf = est.enter_context(tc.tile_pool(name="obuf", bufs=3))
        # PSUM banks: acc(3) + sw(1) + row(1) + bc(1) + yv(2) = 8
        import os as _os
        _pc = [int(v) for v in _os.environ.get("P1_PSUM", "3,1,1,1,2").split(",")]
        ps_acc = est.enter_context(tc.tile_pool(name="ps_acc", bufs=_pc[0], space="PSUM"))
        ps_sw = est.enter_context(tc.tile_pool(name="ps_sw", bufs=_pc[1], space="PSUM"))
        ps_row = est.enter_context(tc.tile_pool(name="ps_row", bufs=_pc[2], space="PSUM"))
        ps_bc = est.enter_context(tc.tile_pool(name="ps_bc", bufs=_pc[3], space="PSUM"))
        ps_yv = est.enter_context(tc.tile_pool(name="ps_yv", bufs=_pc[4], space="PSUM"))

        # ---- constants ----
        ones_col_f = const.tile([128, 1], f32, name="ones_col_f")
        nc.vector.memset(ones_col_f[:], 1.0)
        ones_col = const.tile([128, 1], f32r, name="ones_col")
        nc.scalar.copy(ones_col[:], ones_col_f[:])
        ones_row_f = const.tile([1, 128], f32, name="ones_row_f")
        nc.vector.memset(ones_row_f[:], 1.0)
        ones_row = const.tile([1, 128], f32r, name="ones_row")
        nc.scalar.copy(ones_row[:], ones_row_f[:])
        psw = const.tile([128, 128], f32r, name="psw")
        nc.scalar.dma_start(psw[:], pswap[:])
        pid = const.tile([128, 128], f32r, name="pid")
        nc.scalar.dma_start(pid[:], pident[:])
        hm = const.tile([128, 2], f32r, name="hm")
        nc.scalar.dma_start(hm[:], hmask[:])
        hs = const.tile([2, 128], f32r, name="hs")
        nc.scalar.dma_start(hs[:], hsel[:])

        cs = ropes.tile([128, T], f32, name="cs")
        nc.scalar.dma_start(cs[:], cosR[:])
        ss = ropes.tile([128, T], f32, name="ss")
        nc.scalar.dma_start(ss[:], sinR[:])

        # weights: chunk tiles [128 feat, out-cols]
        wq_t = [wpool.tile([128, 256], f32r, name=f"wq{i}") for i in range(8)]
        wk_t = [wpool.tile([128, 128], f32r, name=f"wk{i}") for i in range(8)]
        wv_t = [wpool.tile([128, 128], f32r, name=f"wv{i}") for i in range(8)]
        wo_t = [wpool.tile([128, C], f32r, name=f"wo{i}") for i in range(2)]
        for i in range(8):
            nc.scalar.dma_start(wq_t[i][:], wqT[bass.ts(i, 128), :])
            nc.scalar.dma_start(wk_t[i][:], wkT[bass.ts(i, 128), :])
            nc.scalar.dma_start(wv_t[i][:], wvT[bass.ts(i, 128), :])
        for i in range(2):
            nc.scalar.dma_start(wo_t[i][:], woT[bass.ts(i, 128), :])

        qT = [qkv.tile([128, T], f32r, name=f"qT{d}") for d in range(2)]
        kT = qkv.tile([128, T], f32r, name="kT")
        vaug = [vap.tile([128, 2, HD + 1], f32r, name=f"va{s}")
                for s in range(NSLOT)]
        onescols = const.tile([128, 2, 1], f32, name="onescols")
        nc.vector.memset(onescols[:], 1.0)
        for s in range(NSLOT):
            nc.gpsimd.tensor_copy(vaug[s][:, :, HD:HD + 1], onescols[:])
        dmask = const.tile([128, 128], f32, name="dmask")
        nc.gpsimd.memset(dmask[:], 1.0)
        nc.gpsimd.affine_select(dmask[:], dmask[:], pattern=[[1, 128]],
                                base=0, channel_multiplier=-1,
                                compare_op=mybir.AluOpType.is_ge, fill=0.0)

        # ---- helper: rope + per-head rmsnorm from a raw projection psum ----
        # rope is norm-preserving per head, so the rms reduction runs on the
        # PRE-rope projection, in parallel with the rope chain.
        def rope_norm(prj, g, out_ap):
            gsl = bass.ts(g, SG)
            # chain A: per-head rms from pre-rope values
            sq = work.tile([128, SG], f32r, tag="sq2", name="sq2")
            nc.scalar.activation(sq[:], prj[:], AF.Square)
            ssq2 = ps_row.tile([2, SG], f32, tag="r1", name="ssq2")
            nc.tensor.matmul(ssq2[:], hm[:], sq[:], start=True, stop=True)
            sr2 = rows.tile([2, SG], f32, tag="sr2", name="sr2")
            nc.scalar.activation(sr2[:], ssq2[:], AF.Sqrt,
                                 bias=EPS, scale=1.0 / HD)
            rr2 = rows.tile([2, SG], f32r, tag="rr2", name="rr2")
            with nc.allow_low_precision(reason="rms bcast rows"):
                nc.vector.reciprocal(rr2[:], sr2[:])
            bch = ps_bc.tile([128, SG], f32, tag="bc", name="bch")
            nc.tensor.matmul(bch[:], hs[:], rr2[:], start=True, stop=True)
            # chain B: rope
            psb = work.tile([128, SG], f32r, tag="psb", name="psb")
            nc.scalar.copy(psb[:], prj[:])
            t1 = work.tile([128, SG], f32, tag="t1", name="t1")
            nc.vector.tensor_mul(t1[:], prj[:], cs[:, gsl])
            swp = ps_sw.tile([128, SG], f32, tag="sw", name="swp_ps")
            nc.tensor.matmul(swp[:], psw[:], psb[:], start=True, stop=True)
            t2 = work.tile([128, SG], f32, tag="t2", name="t2")
            nc.vector.tensor_mul(t2[:], swp[:], ss[:, gsl])
            rq = work.tile([128, SG], f32, tag="rq", name="rq")
            nc.vector.tensor_add(rq[:], t1[:], t2[:])
            nc.vector.tensor_mul(out_ap, rq[:], bch[:])

        # ================= projections, by token group =================
        for g in range(NG):
            gsl = bass.ts(g, SG)
            xg = [xin.tile([128, SG], f32r, tag=f"x{i}", name=f"x{i}")
                  for i in range(8)]
            for i in range(8):
                nc.sync.dma_start(xg[i][:], xT[bass.ts(i, 128), gsl])
            # rms stats of raw x (only V needs the 1/rms scale)
            ssq = ps_row.tile([1, SG], f32, tag="r1", name="ssq")
            for i in range(8):
                sqt = work.tile([128, SG], f32r, tag=f"sqs{i % 2}",
                                name=f"sqs{i}", bufs=2)
                eng = nc.vector if i % 2 == 0 else nc.gpsimd
                eng.tensor_mul(sqt[:], xg[i][:].bitcast(f32),
                               xg[i][:].bitcast(f32))
                nc.tensor.matmul(ssq[:], ones_col[:], sqt[:],
                                 start=(i == 0), stop=(i == 7))
            srow = rows.tile([1, SG], f32, tag="srow", name="srow")
            nc.scalar.activation(srow[:], ssq[:], AF.Sqrt,
                                 bias=EPS, scale=1.0 / C)
            rrow = rows.tile([1, SG], f32r, tag="rrow", name="rrow")
            with nc.allow_low_precision(reason="rms bcast"):
                nc.vector.reciprocal(rrow[:], srow[:])
            bc = ps_bc.tile([128, SG], f32, tag="bc", name="bc")
            nc.tensor.matmul(bc[:], ones_row[:], rrow[:], start=True, stop=True)
            bcs = work.tile([128, SG], f32, tag="bcs", name="bcs", bufs=2)
            nc.vector.tensor_copy(bcs[:], bc[:])

            # Q projections (2 chunks) and K (1 chunk) from RAW x
            for dq in range(2):
                prj = ps_acc.tile([128, SG], f32, tag="acc", name="q_ps")
                for ci in range(8):
                    nc.tensor.matmul(prj[:], wq_t[ci][:, bass.ts(dq, 128)],
                                     xg[ci], start=(ci == 0), stop=(ci == 7))
                rope_norm(prj, g, qT[dq][:, gsl])
            prj = ps_acc.tile([128, SG], f32, tag="acc", name="k_ps")
            for ci in range(8):
                nc.tensor.matmul(prj[:], wk_t[ci][:], xg[ci],
                                 start=(ci == 0), stop=(ci == 7))
            rope_norm(prj, g, kT[:, gsl])

            # V: feature-major (scaled by 1/rms) then transpose to token-major
            vps = ps_acc.tile([128, SG], f32, tag="acc", name="v_ps")
            for ci in range(8):
                nc.tensor.matmul(vps[:], wv_t[ci][:], xg[ci],
                                 start=(ci == 0), stop=(ci == 7))
            vsb = work.tile([128, SG], f32r, tag="vsb", name="vsb")
            nc.vector.tensor_mul(vsb[:], vps[:], bcs[:])
            for tt in range(4):
                vtp = ps_sw.tile([128, 128], f32r, tag="sw", name="vtp")
                nc.tensor.matmul(vtp[:], vsb[:, bass.ts(tt, 128)], pid[:],
                                 is_transpose=True, start=True, stop=True)
                va3 = vaug[g * 4 + tt]
                nc.vector.tensor_copy(
                    va3[:, :, 0:HD],
                    vtp[:].rearrange("p (h d) -> p h d", d=HD))

        # ================= attention + wo, by query group =================
        # The normalize tail of head hp is emitted AFTER the first score
        # matmuls of head hp+1, so the in-order PE queue never stalls on the
        # (DVE) reciprocal at head boundaries.
        pending = None

        def emit_normalize():
            nonlocal pending
            if pending is None:
                return
            yv, yTd, qb = pending
            pending = None
            ry = rows.tile([1, SG], f32r, tag="ry", name="ry")
            with nc.allow_low_precision(reason="softmax denom"):
                nc.vector.reciprocal(ry[:], yv[HD:HD + 1, :])
            ybc = ps_bc.tile([64, SG], f32, tag="bc", name="ybc")
            nc.tensor.matmul(ybc[:], ones_row[:, 0:64], ry[:],
                             start=True, stop=True)
            ybcs = work.tile([128, SG], f32, tag="ybcs", name="ybcs")
            nc.vector.tensor_copy(ybcs[qb:qb + 64, :], ybc[:])
            nc.vector.tensor_mul(yTd[qb:qb + 64, :],
                                 yv[0:HD, :], ybcs[qb:qb + 64, :])

        for qg in range(4):
            qsl = bass.ts(qg, SG)
            yT = [ypool.tile([128, SG], f32r, tag=f"yT{d}", name=f"yT{d}")
                  for d in range(2)]
            for hp in range(4):
                dq, qb = hp % 2, 64 * (hp // 2)
                yv = ps_yv.tile([HD + 1, SG], f32, tag="yv", name="yv")
                nslots = 4 * qg + 4
                first = True
                # determine last live slot for window
                live = [s for s in range(nslots)
                        if (qg * SG + max(0, 128 * (s - 4 * qg))
                            - (128 * s + 127)) <= window]
                for s in range(nslots):
                    off = max(0, 128 * (s - 4 * qg))
                    cols = SG - off
                    dmin = qg * SG + off - (128 * s + 127)
                    dmax = qg * SG + SG - 1 - 128 * s
                    if dmin > window:
                        continue
                    sps = ps_acc.tile([128, SG], f32, tag="acc", name="sps")
                    nc.tensor.matmul(
                        sps[:, 0:cols],
                        kT[qb:qb + 64, bass.ts(s, 128)],
                        qT[dq][qb:qb + 64, qg * SG + off:(qg + 1) * SG],
                        start=True, stop=True)
                    pT = pexp.tile([128, SG], f32r, tag="pT", name="pT")
                    nc.scalar.activation(pT[:, 0:cols], sps[:, 0:cols],
                                         AF.Exp, scale=0.125)
                    if s >= 4 * qg:
                        # diagonal 128-block causal mask
                        nc.gpsimd.tensor_mul(pT[:, 0:128], pT[:, 0:128],
                                             dmask[:])
                    if dmax > window:
                        # sliding-window lower boundary
                        nc.gpsimd.affine_select(
                            pT[:, 0:cols], pT[:, 0:cols],
                            pattern=[[1, cols]],
                            base=qg * SG + off - 128 * s - window,
                            channel_multiplier=-1,
                            compare_op=mybir.AluOpType.is_le, fill=0.0)
                    nc.tensor.matmul(yv[:, off:SG], vaug[s][:, hp // 2, :],
                                     pT[:, 0:cols],
                                     start=first, stop=(s == live[-1]))
                    first = False
                    if pending is not None:
                        emit_normalize()
                pending = (yv, yT[dq][:], qb)
            emit_normalize()
            # wo partial for this query group
            for co in range(8):
                ops = ps_acc.tile([128, SG], f32, tag="acc", name="o_ps")
                for ci in range(2):
                    nc.tensor.matmul(ops[:], wo_t[ci][:, bass.ts(co, 128)],
                                     yT[ci][:], start=(ci == 0), stop=(ci == 1))
                ob = obuf.tile([128, SG], f32, tag="ob", name="ob")
                if co % 2 == 0:
                    nc.scalar.copy(ob[:], ops[:])
                else:
                    nc.vector.tensor_copy(ob[:], ops[:])
                nc.sync.dma_start(y_out[bass.ts(co, 128), qsl], ob[:])

    nc.compile()
    return nc


# --------------------------------------------------------------------------
# Phase 2: expert MLP, fp8 DoubleRow with hi/lo split quantization
# --------------------------------------------------------------------------
def build_phase2(ncap: int):
    nc = bacc.Bacc("TRN2", target_bir_lowering=False, debug=False,
                   num_devices=NCORES)
    NT = ncap // 512

    xhi = nc.dram_tensor("xhi", [128, 8, ncap], fp8, kind="ExternalInput").ap()
    xlo = nc.dram_tensor("xlo", [128, 8, ncap], fp8, kind="ExternalInput").ap()
    wuph = nc.dram_tensor("wuph", [128, 8, HID], fp8, kind="ExternalInput").ap()
    wuplo = nc.dram_tensor("wuplo", [128, 8, HID], fp8, kind="ExternalInput").ap()
    wdnh = nc.dram_tensor("wdnh", [128, 16, C], fp8, kind="ExternalInput").ap()
    wdnlo = nc.dram_tensor("wdnlo", [128, 16, C], fp8, kind="ExternalInput").ap()
    moe_out = nc.dram_tensor("moe_out", [C, ncap], bf16, kind="ExternalOutput").ap()

    from contextlib import ExitStack
    with tile.TileContext(nc) as tc, ExitStack() as est:
        wpool = est.enter_context(tc.tile_pool(name="wpool", bufs=1))
        hpool = est.enter_context(tc.tile_pool(name="hpool", bufs=2))
        stream = est.enter_context(tc.tile_pool(name="stream", bufs=1))
        work = est.enter_context(tc.tile_pool(name="work", bufs=3))
        ps_h = est.enter_context(tc.tile_pool(name="ps_h", bufs=4, space="PSUM"))
        ps_o = est.enter_context(tc.tile_pool(name="ps_o", bufs=4, space="PSUM"))

        xh_t = stream.tile([128, 8, ncap], fp8, name="xh_t")
        nc.sync.dma_start(xh_t[:], xhi[:])
        xl_t = stream.tile([128, 8, ncap], fp8, name="xl_t")
        nc.sync.dma_start(xl_t[:], xlo[:])

        uph_t = wpool.tile([128, 8, HID], fp8, name="uph_t")
        uplo_t = wpool.tile([128, 8, HID], fp8, name="uplo_t")
        dnh_t = wpool.tile([128, 16, C], fp8, name="dnh_t")
        dnlo_t = wpool.tile([128, 16, C], fp8, name="dnlo_t")
        for j in range(4):
            js = bass.ts(j, HID // 4)
            nc.scalar.dma_start(uph_t[:, :, js], wuph[:, :, js])
            nc.scalar.dma_start(uplo_t[:, :, js], wuplo[:, :, js])
        nc.scalar.dma_start(dnh_t[:], wdnh[:])
        nc.scalar.dma_start(dnlo_t[:], wdnlo[:])

        for nt in range(NT):
            csl = bass.ts(nt, 512)
            hT = hpool.tile([128, 16, 512], fp8, tag="hT", name="hT")
            for hc in range(16):
                h_ps = ps_h.tile([128, 512], f32, tag="h", name="h_ps")
                hsl = bass.ts(hc, 128)
                for c2 in range(4):
                    p2 = slice(2 * c2, 2 * c2 + 2)
                    nc.tensor.matmul(h_ps[:], uph_t[:, p2, hsl],
                                     xh_t[:, p2, csl], start=(c2 == 0),
                                     stop=False, perf_mode=DR)
                for c2 in range(4):
                    p2 = slice(2 * c2, 2 * c2 + 2)
                    nc.tensor.matmul(h_ps[:], uph_t[:, p2, hsl],
                                     xl_t[:, p2, csl], start=False,
                                     stop=False, perf_mode=DR)
                for c2 in range(4):
                    p2 = slice(2 * c2, 2 * c2 + 2)
                    nc.tensor.matmul(h_ps[:], uplo_t[:, p2, hsl],
                                     xh_t[:, p2, csl], start=False,
                                     stop=(c2 == 3), perf_mode=DR)
                hr = work.tile([128, 512], f32, tag="hr", name="hr")
                nc.scalar.activation(hr[:], h_ps[:], AF.Relu, scale=1.0 / WU)
                with nc.allow_low_precision(reason="fp8 hidden"):
                    nc.vector.tensor_mul(hT[:, hc, :], hr[:], hr[:])
            for co in range(8):
                o_ps = ps_o.tile([128, 512], f32, tag="o", name="o_ps")
                osl = bass.ts(co, 128)
                for i2 in range(8):
                    p2 = slice(2 * i2, 2 * i2 + 2)
                    nc.tensor.matmul(o_ps[:], dnh_t[:, p2, osl],
                                     hT[:, p2, :], start=(i2 == 0),
                                     stop=False, perf_mode=DR)
                for i2 in range(8):
                    p2 = slice(2 * i2, 2 * i2 + 2)
                    nc.tensor.matmul(o_ps[:], dnlo_t[:, p2, osl],
                                     hT[:, p2, :], start=False,
                                     stop=(i2 == 7), perf_mode=DR)
                ot = work.tile([128, 512], bf16, tag="ot", name="ot")
                with nc.allow_low_precision(reason="bf16 out"):
                    if co % 2 == 0:
                        nc.scalar.copy(ot[:], o_ps[:])
                    else:
                        nc.vector.tensor_copy(ot[:], o_ps[:])
                nc.sync.dma_start(moe_out[bass.ts(co, 128), csl], ot[:])

    nc.compile()
    return nc


# --------------------------------------------------------------------------
# Host orchestration
# --------------------------------------------------------------------------
def _q_cols(g):
    cols = []
    for h_ in (0, 2, 1, 3):
        h = 4 * g + h_
        cols += list(range(h * HD, (h + 1) * HD))
    return np.array(cols, np.int64)


def _phase1_inputs(x, cos, sin, wq, wk, wv, wo):
    cosT = np.ascontiguousarray(cos[0, :, 0, :].T)  # (32, T)
    sinT = np.ascontiguousarray(sin[0, :, 0, :].T)
    cosR = np.tile(cosT, (4, 1)).astype(np.float32)
    sinR = np.tile(np.vstack([sinT, -sinT]), (2, 1)).astype(np.float32)

    psw = np.zeros((128, 128), np.float32)
    psw[np.arange(128) ^ 32, np.arange(128)] = 1.0
    pident = np.eye(128, dtype=np.float32)
    hmask = np.zeros((128, 2), np.float32)
    hmask[0:64, 0] = 1.0
    hmask[64:128, 1] = 1.0
    hsel = np.ascontiguousarray(hmask.T)

    xTb = [np.ascontiguousarray(x[b].T) for b in range(B)]
    in_maps = []
    for c in range(NCORES):
        b, g = c // 4, c % 4
        qc = _q_cols(g)
        kvs = slice(128 * g, 128 * g + 128)
        in_maps.append(dict(
            xT=xTb[b], cosR=cosR, sinR=sinR,
            wqT=np.ascontiguousarray(wq.T[:, qc]),
            wkT=np.ascontiguousarray(wk.T[:, kvs]),
            wvT=np.ascontiguousarray(wv.T[:, kvs]),
            woT=np.ascontiguousarray(wo.T[qc, :]),
            pswap=psw, pident=pident, hmask=hmask, hsel=hsel,
        ))
    return in_maps


def _route(logits, router_bias):
    sig = (1.0 / (1.0 + np.exp(-logits.astype(np.float32)))).astype(np.float32)
    sel = sig + router_bias[None, :].astype(np.float32)
    idx = np.argsort(-sel, axis=1, kind="stable")[:, :TOPK]
    tw = np.take_along_axis(sig, idx, axis=1)
    tw = tw / tw.sum(axis=1, keepdims=True)
    N = logits.shape[0]
    sparse_w = np.zeros((N, E_MLP + E_VE), np.float32)
    np.put_along_axis(sparse_w, idx, tw, axis=1)
    return sparse_w


def _fold(a, nchunk):
    a = np.asarray(a)
    return np.ascontiguousarray(
        a.reshape(nchunk, 128, a.shape[-1]).transpose(1, 0, 2))


def kernel(**inputs):
    x = np.asarray(inputs["x"], np.float32)
    token_ids = np.asarray(inputs["token_ids"])
    cos = np.asarray(inputs["cos"], np.float32)
    sin = np.asarray(inputs["sin"], np.float32)
    window = int(np.asarray(inputs["window_size"]))
    wq, wk, wv, wo = (np.asarray(inputs[k], np.float32)
                      for k in ("wq", "wk", "wv", "wo"))
    w_up = np.asarray(inputs["w_up"], np.float32)
    w_down = np.asarray(inputs["w_down"], np.float32)
    router_w = np.asarray(inputs["router_w"], np.float32)
    router_bias = np.asarray(inputs["router_bias"], np.float32)
    ve_tables = np.asarray(inputs["ve_tables"], np.float32)

    key1 = ("p1", window)
    if key1 not in _prog_cache:
        _prog_cache[key1] = build_phase1(window)
    nc1 = _prog_cache[key1]

    in_maps = _phase1_inputs(x, cos, sin, wq, wk, wv, wo)
    res1 = run_bass_kernel_spmd(nc1, in_maps, list(range(NCORES))).results

    # host: reduce partials, residual, rmsnorm, router
    x2 = np.empty((B, T, C), np.float32)
    for b in range(B):
        acc = x[b].T.copy()
        for g in range(4):
            acc += res1[4 * b + g]["y_out"]
        x2[b] = acc.T
    x2f = x2.reshape(-1, C)
    xf = x2f / np.sqrt((x2f * x2f).mean(1, keepdims=True) + EPS)
    xf = xf.astype(np.float32)
    logits = xf @ router_w.T
    sparse_w = _route(logits, router_bias)

    # dispatch
    ncap = NCAP
    idx_list, n_list = [], []
    for e in range(E_MLP):
        idx_e = np.nonzero(sparse_w[:, e])[0]
        idx_list.append(idx_e)
        n_list.append(len(idx_e))
    max_n = max(n_list)
    while ncap < max_n:
        ncap *= 2

    key2 = ("p2", ncap)
    if key2 not in _prog_cache:
        _prog_cache[key2] = build_phase2(ncap)
    nc2 = _prog_cache[key2]

    xfT = np.ascontiguousarray(xf.T)  # (C, N)
    in_maps2 = []
    for e in range(NCORES):
        idx_e = idx_list[e]
        n_e = n_list[e]
        xe = np.zeros((C, ncap), np.float32)
        xe[:, :n_e] = xfT[:, idx_e]
        xh = xe.astype(E4NP)
        xl = (xe - xh.astype(np.float32)).astype(E4NP)
        wu = w_up[e].T * WU          # (C, HID)
        wuh = wu.astype(E4NP)
        wul = (wu - wuh.astype(np.float32)).astype(E4NP)
        wd = w_down[e].T * WD        # (HID, C)
        wdh = wd.astype(E4NP)
        wdl = (wd - wdh.astype(np.float32)).astype(E4NP)
        in_maps2.append(dict(
            xhi=_fold(xh, 8), xlo=_fold(xl, 8),
            wuph=_fold(wuh, 8), wuplo=_fold(wul, 8),
            wdnh=_fold(wdh, 16), wdnlo=_fold(wdl, 16)))
    res2 = run_bass_kernel_spmd(nc2, in_maps2, list(range(NCORES))).results

    out = x2f.copy()
    # VE experts on host (pure gather + scale)
    tok = token_ids.reshape(-1)
    for ee in range(E_VE):
        w = sparse_w[:, E_MLP + ee]
        nz = np.nonzero(w)[0]
        out[nz] += w[nz, None] * ve_tables[ee][tok[nz]]
    for e in range(E_MLP):
        n_e = n_list[e]
        if n_e:
            moe = res2[e]["moe_out"][:, :n_e]
            g = (sparse_w[idx_list[e], e] / WD).astype(np.float32)
            out[idx_list[e]] += g[:, None] * moe.astype(np.float32).T
    return out.reshape(B, T, C).astype(np.float32)


# revision 4
# speedup vs baseline: 1.1287x; 1.0100x over previous
"""TRN2 Bass kernel v2 for nn_BlockMoVaE — head-sharded attention.

Sharding:
  Phase 1 (attention): core c = (b, g) with b = c//4, g = c%4 computes heads
    {4g..4g+3} (kv heads {2g, 2g+1}) of batch b over ALL T=2048 tokens, and
    outputs the wo-projected partial y contribution [C, T].  Each core sees
    the full causal triangle, so no slot is dead and total score/AV work is
    halved vs token-sharding.  Host reduces the 4 partials per batch, adds
    the residual, computes rmsnorm + router logits (tiny O(N*C) work) and
    does top-2 routing exactly as the reference.
  Phase 2 (expert MLP): core e runs expert e over its routed tokens
    (capacity-padded).  fp8(e4m3) weights/activations with DoubleRow
    matmuls; both weight matrices and the up-proj input are SPLIT into
    hi + lo fp8 pairs (3-pass up, 2-pass down) so the only surviving
    quantization error is the fp8 hidden state (~1e-2 of output max).
    VE (vocab-embedding) experts are a pure host-side gather + scale.

Key identity: rmsnorm(rope(rmsnorm(x) @ wq)) == rmsnorm(rope(x @ wq)) per
head, because the per-token rms scale commutes through the projection and
rope and cancels in the final per-head rmsnorm.  So Q/K are projected from
RAW x; only V needs the per-token 1/rms scale (applied on the psum copy).
"""
import numpy as np
import ml_dtypes

import concourse.bass as bass
import concourse.bacc as bacc
import concourse.mybir as mybir
import concourse.tile as tile
from concourse.bass_utils import run_bass_kernel_spmd

# ---- problem constants ----
B, T, C = 2, 2048, 1024
NH, NKV, HD = 16, 8, 64
E_MLP, E_VE, TOPK = 8, 2, 2
HID = 2048
VOCAB = 50257
EPS = 1e-6
NCORES = 8
NSLOT = T // 128      # 16
NG = 4                # token groups of 512 in phase 1
SG = T // NG          # 512
NCAP = 1024           # phase-2 expert capacity
WU = 64.0             # fp8 up-weight pre-scale
WD = 64.0             # fp8 down-weight pre-scale

f32 = mybir.dt.float32
f32r = mybir.dt.float32r
bf16 = mybir.dt.bfloat16
fp8 = mybir.dt.float8e4
E4NP = ml_dtypes.float8_e4m3
BF16NP = ml_dtypes.bfloat16
AF = mybir.ActivationFunctionType
DR = mybir.MatmulPerfMode.DoubleRow

_prog_cache = {}


def _register_consts(nc, values):
    for value in values:
        key = (f32, float(value))
        if key not in nc.const_aps.aps:
            t = nc.alloc_sbuf_tensor(f"constap-{value}", [128, 1], f32)
            nc.gpsimd.memset(t.ap(), float(value))
            nc.const_aps.aps[key] = t.ap()
    nc.all_engine_barrier()


# --------------------------------------------------------------------------
# Phase 1: head-sharded attention, partial wo output
# --------------------------------------------------------------------------
def build_phase1(window: int):
    nc = bacc.Bacc("TRN2", target_bir_lowering=False, debug=False,
                   num_devices=NCORES)

    xT = nc.dram_tensor("xT", [128, 8, T], f32r, kind="ExternalInput").ap()
    cosR = nc.dram_tensor("cosR", [128, T], f32, kind="ExternalInput").ap()
    sinR = nc.dram_tensor("sinR", [128, T], f32, kind="ExternalInput").ap()
    wqT = nc.dram_tensor("wqT", [128, 8, 256], f32r, kind="ExternalInput").ap()
    wkT = nc.dram_tensor("wkT", [128, 8, 128], f32r, kind="ExternalInput").ap()
    wvT = nc.dram_tensor("wvT", [128, 8, 128], f32r, kind="ExternalInput").ap()
    woT = nc.dram_tensor("woT", [128, 2, C], f32r, kind="ExternalInput").ap()
    pswap = nc.dram_tensor("pswap", [128, 128], f32r, kind="ExternalInput").ap()
    pident = nc.dram_tensor("pident", [128, 128], f32r, kind="ExternalInput").ap()
    hmask = nc.dram_tensor("hmask", [128, 2], f32r, kind="ExternalInput").ap()
    hsel = nc.dram_tensor("hsel", [2, 128], f32r, kind="ExternalInput").ap()

    y_out = nc.dram_tensor("y_out", [C, T], f32, kind="ExternalOutput").ap()

    _register_consts(nc, [EPS, 0.0])
    from contextlib import ExitStack
    with tile.TileContext(nc) as tc, ExitStack() as est:
        const = est.enter_context(tc.tile_pool(name="const", bufs=1))
        wpool = est.enter_context(tc.tile_pool(name="wpool", bufs=1))
        ropes = est.enter_context(tc.tile_pool(name="ropes", bufs=2))
        xin = est.enter_context(tc.tile_pool(name="xin", bufs=2))
        qkv = est.enter_context(tc.tile_pool(name="qkv", bufs=1))
        vap = est.enter_context(tc.tile_pool(name="vap", bufs=1))
        work = est.enter_context(tc.tile_pool(name="work", bufs=2))
        rows = est.enter_context(tc.tile_pool(name="rows", bufs=2))
        pexp = est.enter_context(tc.tile_pool(name="pexp", bufs=4))
        ypool = est.enter_context(tc.tile_pool(name="ypool", bufs=2))
        obuf = est.enter_context(tc.tile_pool(name="obuf", bufs=3))
        # PSUM banks: acc(3) + sw(1) + row(1) + bc(1) + yv(2) = 8
        ps_acc = est.enter_context(tc.tile_pool(name="ps_acc", bufs=3, space="PSUM"))
        ps_sw = est.enter_context(tc.tile_pool(name="ps_sw", bufs=1, space="PSUM"))
        ps_row = est.enter_context(tc.tile_pool(name="ps_row", bufs=1, space="PSUM"))
        ps_bc = est.enter_context(tc.tile_pool(name="ps_bc", bufs=1, space="PSUM"))
        ps_yv = est.enter_context(tc.tile_pool(name="ps_yv", bufs=2, space="PSUM"))

        # ---- constants ----
        ones_col_f = const.tile([128, 1], f32, name="ones_col_f")
        nc.vector.memset(ones_col_f[:], 1.0)
        ones_col = const.tile([128, 1], f32r, name="ones_col")
        nc.scalar.copy(ones_col[:], ones_col_f[:])
        ones_row_f = const.tile([1, 128], f32, name="ones_row_f")
        nc.vector.memset(ones_row_f[:], 1.0)
        ones_row = const.tile([1, 128], f32r, name="ones_row")
        nc.scalar.copy(ones_row[:], ones_row_f[:])
        # x group 0 + rope-table slices go FIRST on the (single) SP DMA
        # queue so group-0 compute starts immediately; weights follow.
        xqs = [None] * NG
        css = [None] * NG
        sss = [None] * NG

        def load_group(g):
            xqs[g] = xin.tile([128, 8, SG], f32r, tag="x", name="xq")
            nc.sync.dma_start(xqs[g][:], xT[:, :, bass.ts(g, SG)])
            css[g] = ropes.tile([128, SG], f32, tag="cs", name="cs")
            nc.sync.dma_start(css[g][:], cosR[:, bass.ts(g, SG)])
            sss[g] = ropes.tile([128, SG], f32, tag="ss", name="ss")
            nc.sync.dma_start(sss[g][:], sinR[:, bass.ts(g, SG)])

        load_group(0)
        psw = const.tile([128, 128], f32r, name="psw")
        nc.sync.dma_start(psw[:], pswap[:])
        hm = const.tile([128, 2], f32r, name="hm")
        nc.sync.dma_start(hm[:], hmask[:])
        hs = const.tile([2, 128], f32r, name="hs")
        nc.sync.dma_start(hs[:], hsel[:])
        wq_t = wpool.tile([128, 8, 256], f32r, name="wq_t")
        wk_t = wpool.tile([128, 8, 128], f32r, name="wk_t")
        wv_t = wpool.tile([128, 8, 128], f32r, name="wv_t")
        wo_t = wpool.tile([128, 2, C], f32r, name="wo_t")
        nc.sync.dma_start(wq_t[:], wqT[:])
        nc.sync.dma_start(wk_t[:], wkT[:])
        nc.sync.dma_start(wv_t[:], wvT[:])
        pid = const.tile([128, 128], f32r, name="pid")
        nc.sync.dma_start(pid[:], pident[:])

        qT = [qkv.tile([128, T], f32r, name=f"qT{d}") for d in range(2)]
        kT = qkv.tile([128, T], f32r, name="kT")
        vaug = [vap.tile([128, 2, HD + 1], f32r, name=f"va{s}")
                for s in range(NSLOT)]
        onescols = const.tile([128, 2, 1], f32, name="onescols")
        nc.vector.memset(onescols[:], 1.0)
        for s in range(NSLOT):
            nc.gpsimd.tensor_copy(vaug[s][:, :, HD:HD + 1], onescols[:])
        dmask = const.tile([128, 128], f32, name="dmask")
        nc.gpsimd.memset(dmask[:], 1.0)
        nc.gpsimd.affine_select(dmask[:], dmask[:], pattern=[[1, 128]],
                                base=0, channel_multiplier=-1,
                                compare_op=mybir.AluOpType.is_ge, fill=0.0)

        # ---- software-pipelined projections ----
        # Each chunk's cross-engine tail (swap matmul, rms reduce, rope
        # vector chain) is deferred into the NEXT chunk's projection-matmul
        # stream via a FIFO of closures, so the in-order PE queue never
        # stalls waiting for Act/DVE results.
        pend = []

        def pump():
            if pend:
                pend.pop(0)()

        def proj_mms(prj, lhsT_of, xg, extra=None):
            for ci in range(8):
                nc.tensor.matmul(prj[:], lhsT_of(ci), xg[ci],
                                 start=(ci == 0), stop=(ci == 7))
                if extra is not None and ci < len(extra):
                    extra[ci]()
                elif ci in (2, 4, 6):
                    pump()

        def rope_norm(prj, g, out_ap):
            gsl = bass.ts(g, SG)
            sq = work.tile([128, SG], f32r, tag="sq2", name="sq2")
            nc.scalar.activation(sq[:], prj[:], AF.Square)
            psb = work.tile([128, SG], f32r, tag="psb", name="psb")
            nc.scalar.copy(psb[:], prj[:])
            t1 = work.tile([128, SG], f32, tag="t1", name="t1")
            nc.vector.tensor_mul(t1[:], prj[:], css[g][:])

            def tail1():
                ssq2 = ps_row.tile([2, SG], f32, tag="r1", name="ssq2")
                nc.tensor.matmul(ssq2[:], hm[:], sq[:], start=True, stop=True)
                swp = ps_sw.tile([128, SG], f32, tag="sw", name="swp_ps")
                nc.tensor.matmul(swp[:], psw[:], psb[:], start=True, stop=True)
                sr2 = rows.tile([2, SG], f32, tag="sr2", name="sr2")
                nc.scalar.activation(sr2[:], ssq2[:], AF.Sqrt,
                                     bias=EPS, scale=1.0 / HD)
                rr2 = rows.tile([2, SG], f32r, tag="rr2", name="rr2")
                with nc.allow_low_precision(reason="rms bcast rows"):
                    nc.vector.reciprocal(rr2[:], sr2[:])
                t2 = work.tile([128, SG], f32, tag="t2", name="t2")
                nc.vector.tensor_mul(t2[:], swp[:], sss[g][:])
                rq = work.tile([128, SG], f32, tag="rq", name="rq")
                nc.vector.tensor_add(rq[:], t1[:], t2[:])

                def tail2():
                    bch = ps_bc.tile([128, SG], f32, tag="bc", name="bch")
                    nc.tensor.matmul(bch[:], hs[:], rr2[:],
                                     start=True, stop=True)
                    nc.vector.tensor_mul(out_ap, rq[:], bch[:])
                pend.append(tail2)
            pend.append(tail1)

        # ================= projections, by token group =================
        for g in range(NG):
            gsl = bass.ts(g, SG)
            if g + 1 < NG:
                load_group(g + 1)
            if g == 1:
                nc.sync.dma_start(wo_t[:], woT[:])
            xg = [xqs[g][:, i, :] for i in range(8)]
            # rms stats of raw x (only V needs the 1/rms scale)
            sqts = []
            for i in range(8):
                sqt = work.tile([128, SG], f32r, tag=f"sqs{i % 2}",
                                name=f"sqs{i}", bufs=2)
                nc.gpsimd.tensor_mul(sqt[:], xg[i].bitcast(f32),
                                     xg[i].bitcast(f32))
                sqts.append(sqt)
            ssq = ps_row.tile([1, SG], f32, tag="r1", name="ssq")

            def mk_ssq(i):
                def f():
                    nc.tensor.matmul(ssq[:], ones_col[:], sqts[i][:],
                                     start=(i == 0), stop=(i == 7))
                return f

            # Q0: interleave the 8 group-rms reduce matmuls 1:1
            prj = ps_acc.tile([128, SG], f32, tag="acc", name="q_ps")
            proj_mms(prj, lambda ci: wq_t[:, ci, 0:128], xg,
                     extra=[mk_ssq(i) for i in range(8)])
            rope_norm(prj, g, qT[0][:, gsl])
            srow = rows.tile([1, SG], f32, tag="srow", name="srow")
            nc.scalar.activation(srow[:], ssq[:], AF.Sqrt,
                                 bias=EPS, scale=1.0 / C)
            rrow = rows.tile([1, SG], f32r, tag="rrow", name="rrow")
            with nc.allow_low_precision(reason="rms bcast"):
                nc.vector.reciprocal(rrow[:], srow[:])

            # Q1, K
            prj = ps_acc.tile([128, SG], f32, tag="acc", name="q_ps")
            proj_mms(prj, lambda ci: wq_t[:, ci, 128:256], xg)
            rope_norm(prj, g, qT[1][:, gsl])
            prj = ps_acc.tile([128, SG], f32, tag="acc", name="k_ps")
            proj_mms(prj, lambda ci: wk_t[:, ci, :], xg)
            rope_norm(prj, g, kT[:, gsl])

            # V: feature-major (scaled by 1/rms) then transpose to token-major
            vps = ps_acc.tile([128, SG], f32, tag="acc", name="v_ps")
            proj_mms(vps, lambda ci: wv_t[:, ci, :], xg)

            def mk_vtail(g, vps, rrow):
                def f():
                    bc = ps_bc.tile([128, SG], f32, tag="bc", name="bc")
                    nc.tensor.matmul(bc[:], ones_row[:], rrow[:],
                                     start=True, stop=True)
                    bcs = work.tile([128, SG], f32, tag="bcs", name="bcs")
                    nc.vector.tensor_copy(bcs[:], bc[:])
                    vsb = work.tile([128, SG], f32r, tag="vsb", name="vsb")
                    nc.vector.tensor_mul(vsb[:], vps[:], bcs[:])

                    def tr(tt0):
                        def h():
                            for tt in (tt0, tt0 + 1):
                                vtp = ps_sw.tile([128, 128], f32r, tag="sw",
                                                 name="vtp")
                                nc.tensor.matmul(vtp[:],
                                                 vsb[:, bass.ts(tt, 128)],
                                                 pid[:], is_transpose=True,
                                                 start=True, stop=True)
                                va3 = vaug[g * 4 + tt]
                                nc.vector.tensor_copy(
                                    va3[:, :, 0:HD],
                                    vtp[:].rearrange("p (h d) -> p h d", d=HD))
                        return h
                    pend.append(tr(0))
                    pend.append(tr(2))
                return f
            pend.append(mk_vtail(g, vps, rrow))
        while pend:
            pump()

        # ================= attention + wo, by query group =================
        # The normalize tail of head hp is emitted AFTER the first score
        # matmuls of head hp+1, so the in-order PE queue never stalls on the
        # (DVE) reciprocal at head boundaries.
        pending = None

        def emit_normalize():
            nonlocal pending
            if pending is None:
                return
            yv, yTd, qb = pending
            pending = None
            ry = rows.tile([1, SG], f32r, tag="ry", name="ry")
            with nc.allow_low_precision(reason="softmax denom"):
                nc.vector.reciprocal(ry[:], yv[HD:HD + 1, :])
            ybc = ps_bc.tile([64, SG], f32, tag="bc", name="ybc")
            nc.tensor.matmul(ybc[:], ones_row[:, 0:64], ry[:],
                             start=True, stop=True)
            ybcs = work.tile([128, SG], f32, tag="ybcs", name="ybcs")
            nc.vector.tensor_copy(ybcs[qb:qb + 64, :], ybc[:])
            nc.vector.tensor_mul(yTd[qb:qb + 64, :],
                                 yv[0:HD, :], ybcs[qb:qb + 64, :])

        # scores run AHEAD=2 slots before the matching yv so the in-order PE
        # queue never stalls on the Act-engine exp.
        AHEAD = 2
        for qg in range(4):
            qsl = bass.ts(qg, SG)
            yT = [ypool.tile([128, SG], f32r, tag=f"yT{d}", name=f"yT{d}")
                  for d in range(2)]
            for hp in range(4):
                dq, qb = hp % 2, 64 * (hp // 2)
                yv = ps_yv.tile([HD + 1, SG], f32, tag="yv", name="yv")
                nslots = 4 * qg + 4
                live = [s for s in range(nslots)
                        if (qg * SG + max(0, 128 * (s - 4 * qg))
                            - (128 * s + 127)) <= window]
                pTs = {}

                def emit_score(s):
                    off = max(0, 128 * (s - 4 * qg))
                    cols = SG - off
                    dmax = qg * SG + SG - 1 - 128 * s
                    sps = ps_acc.tile([128, SG], f32, tag="acc", name="sps")
                    nc.tensor.matmul(
                        sps[:, 0:cols],
                        kT[qb:qb + 64, bass.ts(s, 128)],
                        qT[dq][qb:qb + 64, qg * SG + off:(qg + 1) * SG],
                        start=True, stop=True)
                    pT = pexp.tile([128, SG], f32r, tag="pT", name="pT")
                    nc.scalar.activation(pT[:, 0:cols], sps[:, 0:cols],
                                         AF.Exp, scale=0.125)
                    if s >= 4 * qg:
                        nc.gpsimd.tensor_mul(pT[:, 0:128], pT[:, 0:128],
                                             dmask[:])
                    if dmax > window:
                        nc.gpsimd.affine_select(
                            pT[:, 0:cols], pT[:, 0:cols],
                            pattern=[[1, cols]],
                            base=qg * SG + off - 128 * s - window,
                            channel_multiplier=-1,
                            compare_op=mybir.AluOpType.is_le, fill=0.0)
                    pTs[s] = (pT, off, cols)

                for j in range(min(AHEAD, len(live))):
                    emit_score(live[j])
                for idx, s in enumerate(live):
                    if idx + AHEAD < len(live):
                        emit_score(live[idx + AHEAD])
                    pT, off, cols = pTs.pop(s)
                    nc.tensor.matmul(yv[:, off:SG], vaug[s][:, hp // 2, :],
                                     pT[:, 0:cols],
                                     start=(idx == 0), stop=(s == live[-1]))
                    if idx == 1 and pending is not None:
                        emit_normalize()
                if pending is not None:
                    emit_normalize()
                pending = (yv, yT[dq][:], qb)
            emit_normalize()
            # wo partial for this query group (stores batched 4 chunks/DMA)
            yof = y_out.rearrange("(a p) t -> p a t", p=128)
            for co in range(8):
                ops = ps_acc.tile([128, SG], f32, tag="acc", name="o_ps")
                for ci in range(2):
                    nc.tensor.matmul(ops[:], wo_t[:, ci, bass.ts(co, 128)],
                                     yT[ci][:], start=(ci == 0), stop=(ci == 1))
                if co % 4 == 0:
                    ob = obuf.tile([128, 4, SG], f32, tag="ob", name="ob")
                if co % 4 == 0:
                    nc.scalar.copy(ob[:, 0, :], ops[:])
                else:
                    nc.vector.tensor_copy(ob[:, co % 4, :], ops[:])
                if co % 4 == 3:
                    nc.sync.dma_start(yof[:, co - 3:co + 1, qsl], ob[:])

    nc.compile()
    return nc


# --------------------------------------------------------------------------
# Phase 2: expert MLP, fp8 DoubleRow with hi/lo split quantization
# --------------------------------------------------------------------------
def build_phase2(ncap: int):
    nc = bacc.Bacc("TRN2", target_bir_lowering=False, debug=False,
                   num_devices=NCORES)
    NT = ncap // 512

    xhi = nc.dram_tensor("xhi", [128, 8, ncap], fp8, kind="ExternalInput").ap()
    xlo = nc.dram_tensor("xlo", [128, 8, ncap], fp8, kind="ExternalInput").ap()
    wuph = nc.dram_tensor("wuph", [128, 8, HID], fp8, kind="ExternalInput").ap()
    wuplo = nc.dram_tensor("wuplo", [128, 8, HID], fp8, kind="ExternalInput").ap()
    wdnh = nc.dram_tensor("wdnh", [128, 16, C], fp8, kind="ExternalInput").ap()
    wdnlo = nc.dram_tensor("wdnlo", [128, 16, C], fp8, kind="ExternalInput").ap()
    moe_out = nc.dram_tensor("moe_out", [C, ncap], bf16, kind="ExternalOutput").ap()

    from contextlib import ExitStack
    with tile.TileContext(nc) as tc, ExitStack() as est:
        wpool = est.enter_context(tc.tile_pool(name="wpool", bufs=1))
        hpool = est.enter_context(tc.tile_pool(name="hpool", bufs=2))
        stream = est.enter_context(tc.tile_pool(name="stream", bufs=1))
        work = est.enter_context(tc.tile_pool(name="work", bufs=3))
        ps_h = est.enter_context(tc.tile_pool(name="ps_h", bufs=4, space="PSUM"))
        ps_o = est.enter_context(tc.tile_pool(name="ps_o", bufs=4, space="PSUM"))

        xh_t = stream.tile([128, 8, ncap], fp8, name="xh_t")
        nc.scalar.dma_start(xh_t[:], xhi[:])
        xl_t = stream.tile([128, 8, ncap], fp8, name="xl_t")
        nc.scalar.dma_start(xl_t[:], xlo[:])

        uph_t = wpool.tile([128, 8, HID], fp8, name="uph_t")
        uplo_t = wpool.tile([128, 8, HID], fp8, name="uplo_t")
        dnh_t = wpool.tile([128, 16, C], fp8, name="dnh_t")
        dnlo_t = wpool.tile([128, 16, C], fp8, name="dnlo_t")
        for j in range(2):
            js = bass.ts(j, HID // 2)
            nc.sync.dma_start(uph_t[:, :, js], wuph[:, :, js])
            nc.sync.dma_start(uplo_t[:, :, js], wuplo[:, :, js])
        nc.sync.dma_start(dnh_t[:], wdnh[:])
        nc.sync.dma_start(dnlo_t[:], wdnlo[:])

        for nt in range(NT):
            csl = bass.ts(nt, 512)
            hT = hpool.tile([128, 16, 512], fp8, tag="hT", name="hT")
            for hc in range(16):
                h_ps = ps_h.tile([128, 512], f32, tag="h", name="h_ps")
                hsl = bass.ts(hc, 128)
                for c2 in range(4):
                    p2 = slice(2 * c2, 2 * c2 + 2)
                    nc.tensor.matmul(h_ps[:], uph_t[:, p2, hsl],
                                     xh_t[:, p2, csl], start=(c2 == 0),
                                     stop=False, perf_mode=DR)
                for c2 in range(4):
                    p2 = slice(2 * c2, 2 * c2 + 2)
                    nc.tensor.matmul(h_ps[:], uph_t[:, p2, hsl],
                                     xl_t[:, p2, csl], start=False,
                                     stop=False, perf_mode=DR)
                for c2 in range(4):
                    p2 = slice(2 * c2, 2 * c2 + 2)
                    nc.tensor.matmul(h_ps[:], uplo_t[:, p2, hsl],
                                     xh_t[:, p2, csl], start=False,
                                     stop=(c2 == 3), perf_mode=DR)
                hr = work.tile([128, 512], f32, tag="hr", name="hr")
                nc.scalar.activation(hr[:], h_ps[:], AF.Relu, scale=1.0 / WU)
                with nc.allow_low_precision(reason="fp8 hidden"):
                    nc.vector.tensor_mul(hT[:, hc, :], hr[:], hr[:])
            for co in range(8):
                o_ps = ps_o.tile([128, 512], f32, tag="o", name="o_ps")
                osl = bass.ts(co, 128)
                for i2 in range(8):
                    p2 = slice(2 * i2, 2 * i2 + 2)
                    nc.tensor.matmul(o_ps[:], dnh_t[:, p2, osl],
                                     hT[:, p2, :], start=(i2 == 0),
                                     stop=False, perf_mode=DR)
                for i2 in range(8):
                    p2 = slice(2 * i2, 2 * i2 + 2)
                    nc.tensor.matmul(o_ps[:], dnlo_t[:, p2, osl],
                                     hT[:, p2, :], start=False,
                                     stop=(i2 == 7), perf_mode=DR)
                if co % 4 == 0:
                    ot = work.tile([128, 4, 512], bf16, tag="ot", name="ot")
                with nc.allow_low_precision(reason="bf16 out"):
                    if co % 2 == 0:
                        nc.scalar.copy(ot[:, co % 4, :], o_ps[:])
                    else:
                        nc.vector.tensor_copy(ot[:, co % 4, :], o_ps[:])
                if co % 4 == 3:
                    mof = moe_out.rearrange("(a p) n -> p a n", p=128)
                    nc.sync.dma_start(mof[:, co - 3:co + 1, csl], ot[:])

    nc.compile()
    return nc


# --------------------------------------------------------------------------
# Host orchestration
# --------------------------------------------------------------------------
def _q_cols(g):
    cols = []
    for h_ in (0, 2, 1, 3):
        h = 4 * g + h_
        cols += list(range(h * HD, (h + 1) * HD))
    return np.array(cols, np.int64)


def _phase1_inputs(x, cos, sin, wq, wk, wv, wo):
    cosT = np.ascontiguousarray(cos[0, :, 0, :].T)  # (32, T)
    sinT = np.ascontiguousarray(sin[0, :, 0, :].T)
    cosR = np.tile(cosT, (4, 1)).astype(np.float32)
    sinR = np.tile(np.vstack([sinT, -sinT]), (2, 1)).astype(np.float32)

    psw = np.zeros((128, 128), np.float32)
    psw[np.arange(128) ^ 32, np.arange(128)] = 1.0
    pident = np.eye(128, dtype=np.float32)
    hmask = np.zeros((128, 2), np.float32)
    hmask[0:64, 0] = 1.0
    hmask[64:128, 1] = 1.0
    hsel = np.ascontiguousarray(hmask.T)

    xTb = [_fold(x[b].T, 8) for b in range(B)]
    in_maps = []
    for c in range(NCORES):
        b, g = c // 4, c % 4
        qc = _q_cols(g)
        kvs = slice(128 * g, 128 * g + 128)
        in_maps.append(dict(
            xT=xTb[b], cosR=cosR, sinR=sinR,
            wqT=_fold(wq.T[:, qc], 8),
            wkT=_fold(wk.T[:, kvs], 8),
            wvT=_fold(wv.T[:, kvs], 8),
            woT=_fold(wo.T[qc, :], 2),
            pswap=psw, pident=pident, hmask=hmask, hsel=hsel,
        ))
    return in_maps


def _route(logits, router_bias):
    sig = (1.0 / (1.0 + np.exp(-logits.astype(np.float32)))).astype(np.float32)
    sel = sig + router_bias[None, :].astype(np.float32)
    idx = np.argsort(-sel, axis=1, kind="stable")[:, :TOPK]
    tw = np.take_along_axis(sig, idx, axis=1)
    tw = tw / tw.sum(axis=1, keepdims=True)
    N = logits.shape[0]
    sparse_w = np.zeros((N, E_MLP + E_VE), np.float32)
    np.put_along_axis(sparse_w, idx, tw, axis=1)
    return sparse_w


def _fold(a, nchunk):
    a = np.asarray(a)
    return np.ascontiguousarray(
        a.reshape(nchunk, 128, a.shape[-1]).transpose(1, 0, 2))


def kernel(**inputs):
    x = np.asarray(inputs["x"], np.float32)
    token_ids = np.asarray(inputs["token_ids"])
    cos = np.asarray(inputs["cos"], np.float32)
    sin = np.asarray(inputs["sin"], np.float32)
    window = int(np.asarray(inputs["window_size"]))
    wq, wk, wv, wo = (np.asarray(inputs[k], np.float32)
                      for k in ("wq", "wk", "wv", "wo"))
    w_up = np.asarray(inputs["w_up"], np.float32)
    w_down = np.asarray(inputs["w_down"], np.float32)
    router_w = np.asarray(inputs["router_w"], np.float32)
    router_bias = np.asarray(inputs["router_bias"], np.float32)
    ve_tables = np.asarray(inputs["ve_tables"], np.float32)

    key1 = ("p1", window)
    if key1 not in _prog_cache:
        _prog_cache[key1] = build_phase1(window)
    nc1 = _prog_cache[key1]

    in_maps = _phase1_inputs(x, cos, sin, wq, wk, wv, wo)
    res1 = run_bass_kernel_spmd(nc1, in_maps, list(range(NCORES))).results

    # host: reduce partials, residual, rmsnorm, router
    x2 = np.empty((B, T, C), np.float32)
    for b in range(B):
        acc = x[b].T.copy()
        for g in range(4):
            acc += res1[4 * b + g]["y_out"]
        x2[b] = acc.T
    x2f = x2.reshape(-1, C)
    xf = x2f / np.sqrt((x2f * x2f).mean(1, keepdims=True) + EPS)
    xf = xf.astype(np.float32)
    logits = xf @ router_w.T
    sparse_w = _route(logits, router_bias)

    # dispatch
    ncap = NCAP
    idx_list, n_list = [], []
    for e in range(E_MLP):
        idx_e = np.nonzero(sparse_w[:, e])[0]
        idx_list.append(idx_e)
        n_list.append(len(idx_e))
    max_n = max(n_list)
    while ncap < max_n:
        ncap *= 2

    key2 = ("p2", ncap)
    if key2 not in _prog_cache:
        _prog_cache[key2] = build_phase2(ncap)
    nc2 = _prog_cache[key2]

    xfT = np.ascontiguousarray(xf.T)  # (C, N)
    in_maps2 = []
    for e in range(NCORES):
        idx_e = idx_list[e]
        n_e = n_list[e]
        xe = np.zeros((C, ncap), np.float32)
        xe[:, :n_e] = xfT[:, idx_e]
        xh = xe.astype(E4NP)
        xl = (xe - xh.astype(np.float32)).astype(E4NP)
        wu = w_up[e].T * WU          # (C, HID)
        wuh = wu.astype(E4NP)
        wul = (wu - wuh.astype(np.float32)).astype(E4NP)
        wd = w_down[e].T * WD        # (HID, C)
        wdh = wd.astype(E4NP)
        wdl = (wd - wdh.astype(np.float32)).astype(E4NP)
        in_maps2.append(dict(
            xhi=_fold(xh, 8), xlo=_fold(xl, 8),
            wuph=_fold(wuh, 8), wuplo=_fold(wul, 8),
            wdnh=_fold(wdh, 16), wdnlo=_fold(wdl, 16)))
    res2 = run_bass_kernel_spmd(nc2, in_maps2, list(range(NCORES))).results

    out = x2f.copy()
    # VE experts on host (pure gather + scale)
    tok = token_ids.reshape(-1)
    for ee in range(E_VE):
        w = sparse_w[:, E_MLP + ee]
        nz = np.nonzero(w)[0]
        out[nz] += w[nz, None] * ve_tables[ee][tok[nz]]
    for e in range(E_MLP):
        n_e = n_list[e]
        if n_e:
            moe = res2[e]["moe_out"][:, :n_e]
            g = (sparse_w[idx_list[e], e] / WD).astype(np.float32)
            out[idx_list[e]] += g[:, None] * moe.astype(np.float32).T
    return out.reshape(B, T, C).astype(np.float32)


# revision 5
# speedup vs baseline: 1.1411x; 1.0110x over previous
"""TRN2 Bass kernel v2 for nn_BlockMoVaE — head-sharded attention.

Sharding:
  Phase 1 (attention): core c = (b, g) with b = c//4, g = c%4 computes heads
    {4g..4g+3} (kv heads {2g, 2g+1}) of batch b over ALL T=2048 tokens, and
    outputs the wo-projected partial y contribution [C, T].  Each core sees
    the full causal triangle, so no slot is dead and total score/AV work is
    halved vs token-sharding.  Host reduces the 4 partials per batch, adds
    the residual, computes rmsnorm + router logits (tiny O(N*C) work) and
    does top-2 routing exactly as the reference.
  Phase 2 (expert MLP): core e runs expert e over its routed tokens
    (capacity-padded).  fp8(e4m3) weights/activations with DoubleRow
    matmuls; both weight matrices and the up-proj input are SPLIT into
    hi + lo fp8 pairs (3-pass up, 2-pass down) so the only surviving
    quantization error is the fp8 hidden state (~1e-2 of output max).
    VE (vocab-embedding) experts are a pure host-side gather + scale.

Key identity: rmsnorm(rope(rmsnorm(x) @ wq)) == rmsnorm(rope(x @ wq)) per
head, because the per-token rms scale commutes through the projection and
rope and cancels in the final per-head rmsnorm.  So Q/K are projected from
RAW x; only V needs the per-token 1/rms scale (applied on the psum copy).
"""
import numpy as np
import ml_dtypes

import concourse.bass as bass
import concourse.bacc as bacc
import concourse.mybir as mybir
import concourse.tile as tile
from concourse.bass_utils import run_bass_kernel_spmd

# ---- problem constants ----
B, T, C = 2, 2048, 1024
NH, NKV, HD = 16, 8, 64
E_MLP, E_VE, TOPK = 8, 2, 2
HID = 2048
VOCAB = 50257
EPS = 1e-6
NCORES = 8
NSLOT = T // 128      # 16
NG = 4                # token groups of 512 in phase 1
SG = T // NG          # 512
NCAP = 1024           # phase-2 expert capacity
WU = 64.0             # fp8 up-weight pre-scale
WD = 64.0             # fp8 down-weight pre-scale

f32 = mybir.dt.float32
f32r = mybir.dt.float32r
bf16 = mybir.dt.bfloat16
fp8 = mybir.dt.float8e4
E4NP = ml_dtypes.float8_e4m3
BF16NP = ml_dtypes.bfloat16
AF = mybir.ActivationFunctionType
DR = mybir.MatmulPerfMode.DoubleRow

_prog_cache = {}


def _register_consts(nc, values):
    for value in values:
        key = (f32, float(value))
        if key not in nc.const_aps.aps:
            t = nc.alloc_sbuf_tensor(f"constap-{value}", [128, 1], f32)
            nc.gpsimd.memset(t.ap(), float(value))
            nc.const_aps.aps[key] = t.ap()
    nc.all_engine_barrier()


# --------------------------------------------------------------------------
# Phase 1: head-sharded attention, partial wo output
# --------------------------------------------------------------------------
def build_phase1(window: int):
    nc = bacc.Bacc("TRN2", target_bir_lowering=False, debug=False,
                   num_devices=NCORES)

    xT = nc.dram_tensor("xT", [128, 8, T], f32r, kind="ExternalInput").ap()
    cosR = nc.dram_tensor("cosR", [128, T], f32, kind="ExternalInput").ap()
    sinR = nc.dram_tensor("sinR", [128, T], f32, kind="ExternalInput").ap()
    wqT = nc.dram_tensor("wqT", [128, 8, 256], f32r, kind="ExternalInput").ap()
    wkT = nc.dram_tensor("wkT", [128, 8, 128], f32r, kind="ExternalInput").ap()
    wvT = nc.dram_tensor("wvT", [128, 8, 128], f32r, kind="ExternalInput").ap()
    woT = nc.dram_tensor("woT", [128, 2, C], f32r, kind="ExternalInput").ap()
    pswap = nc.dram_tensor("pswap", [128, 128], f32r, kind="ExternalInput").ap()
    pident = nc.dram_tensor("pident", [128, 128], f32r, kind="ExternalInput").ap()
    hmask = nc.dram_tensor("hmask", [128, 2], f32r, kind="ExternalInput").ap()
    hsel = nc.dram_tensor("hsel", [2, 128], f32r, kind="ExternalInput").ap()

    y_out = nc.dram_tensor("y_out", [C, T], f32, kind="ExternalOutput").ap()

    _register_consts(nc, [EPS, 0.0])
    from contextlib import ExitStack
    with tile.TileContext(nc) as tc, ExitStack() as est:
        const = est.enter_context(tc.tile_pool(name="const", bufs=1))
        wpool = est.enter_context(tc.tile_pool(name="wpool", bufs=1))
        ropes = est.enter_context(tc.tile_pool(name="ropes", bufs=2))
        xin = est.enter_context(tc.tile_pool(name="xin", bufs=2))
        qkv = est.enter_context(tc.tile_pool(name="qkv", bufs=1))
        vap = est.enter_context(tc.tile_pool(name="vap", bufs=1))
        work = est.enter_context(tc.tile_pool(name="work", bufs=2))
        rows = est.enter_context(tc.tile_pool(name="rows", bufs=2))
        pexp = est.enter_context(tc.tile_pool(name="pexp", bufs=4))
        ypool = est.enter_context(tc.tile_pool(name="ypool", bufs=2))
        obuf = est.enter_context(tc.tile_pool(name="obuf", bufs=3))
        # PSUM banks: acc(3) + sw(1) + row(1) + bc(1) + yv(2) = 8
        ps_acc = est.enter_context(tc.tile_pool(name="ps_acc", bufs=3, space="PSUM"))
        ps_sw = est.enter_context(tc.tile_pool(name="ps_sw", bufs=1, space="PSUM"))
        ps_row = est.enter_context(tc.tile_pool(name="ps_row", bufs=1, space="PSUM"))
        ps_bc = est.enter_context(tc.tile_pool(name="ps_bc", bufs=1, space="PSUM"))
        ps_yv = est.enter_context(tc.tile_pool(name="ps_yv", bufs=2, space="PSUM"))

        # ---- constants ----
        ones_col_f = const.tile([128, 1], f32, name="ones_col_f")
        nc.vector.memset(ones_col_f[:], 1.0)
        ones_col = const.tile([128, 1], f32r, name="ones_col")
        nc.scalar.copy(ones_col[:], ones_col_f[:])
        ones_row_f = const.tile([1, 128], f32, name="ones_row_f")
        nc.vector.memset(ones_row_f[:], 1.0)
        ones_row = const.tile([1, 128], f32r, name="ones_row")
        nc.scalar.copy(ones_row[:], ones_row_f[:])
        # x group 0 + rope-table slices go FIRST on the (single) SP DMA
        # queue so group-0 compute starts immediately; weights follow.
        xqs = [None] * NG
        css = [None] * NG
        sss = [None] * NG

        def load_group(g):
            xqs[g] = xin.tile([128, 8, SG], f32r, tag="x", name="xq")
            nc.sync.dma_start(xqs[g][:], xT[:, :, bass.ts(g, SG)])
            css[g] = ropes.tile([128, SG], f32, tag="cs", name="cs")
            nc.sync.dma_start(css[g][:], cosR[:, bass.ts(g, SG)])
            sss[g] = ropes.tile([128, SG], f32, tag="ss", name="ss")
            nc.sync.dma_start(sss[g][:], sinR[:, bass.ts(g, SG)])

        # group-0 startup: first half of x, then Q weights, then the rest,
        # so the first projection matmuls can begin ~4us in.
        wq_t = wpool.tile([128, 8, 256], f32r, name="wq_t")
        wk_t = wpool.tile([128, 8, 128], f32r, name="wk_t")
        wv_t = wpool.tile([128, 8, 128], f32r, name="wv_t")
        wo_t = wpool.tile([128, 2, C], f32r, name="wo_t")
        xqs[0] = xin.tile([128, 8, SG], f32r, tag="x", name="xq")
        nc.sync.dma_start(xqs[0][:, 0:4, :], xT[:, 0:4, 0:SG])
        nc.sync.dma_start(wq_t[:], wqT[:])
        nc.sync.dma_start(xqs[0][:, 4:8, :], xT[:, 4:8, 0:SG])
        nc.sync.dma_start(wk_t[:], wkT[:])
        nc.sync.dma_start(wv_t[:], wvT[:])
        css[0] = ropes.tile([128, SG], f32, tag="cs", name="cs")
        nc.sync.dma_start(css[0][:], cosR[:, 0:SG])
        sss[0] = ropes.tile([128, SG], f32, tag="ss", name="ss")
        nc.sync.dma_start(sss[0][:], sinR[:, 0:SG])
        psw = const.tile([128, 128], f32r, name="psw")
        nc.sync.dma_start(psw[:], pswap[:])
        hm = const.tile([128, 2], f32r, name="hm")
        nc.sync.dma_start(hm[:], hmask[:])
        hs = const.tile([2, 128], f32r, name="hs")
        nc.sync.dma_start(hs[:], hsel[:])
        pid = const.tile([128, 128], f32r, name="pid")
        nc.sync.dma_start(pid[:], pident[:])

        qT = [qkv.tile([128, T], f32r, name=f"qT{d}") for d in range(2)]
        kT = qkv.tile([128, T], f32r, name="kT")
        vaug = [vap.tile([128, 2, HD + 1], f32r, name=f"va{s}")
                for s in range(NSLOT)]
        onescols = const.tile([128, 2, 1], f32, name="onescols")
        nc.vector.memset(onescols[:], 1.0)
        for s in range(NSLOT):
            nc.gpsimd.tensor_copy(vaug[s][:, :, HD:HD + 1], onescols[:])
        dmask = const.tile([128, 128], f32, name="dmask")
        nc.gpsimd.memset(dmask[:], 1.0)
        nc.gpsimd.affine_select(dmask[:], dmask[:], pattern=[[1, 128]],
                                base=0, channel_multiplier=-1,
                                compare_op=mybir.AluOpType.is_ge, fill=0.0)

        # ---- software-pipelined projections ----
        # Each chunk's cross-engine tail (swap matmul, rms reduce, rope
        # vector chain) is deferred into the NEXT chunk's projection-matmul
        # stream via a FIFO of closures, so the in-order PE queue never
        # stalls waiting for Act/DVE results.
        pend = []

        def pump():
            if pend:
                pend.pop(0)()

        def proj_mms(prj, lhsT_of, xg, extra=None):
            for ci in range(8):
                nc.tensor.matmul(prj[:], lhsT_of(ci), xg[ci],
                                 start=(ci == 0), stop=(ci == 7))
                if extra is not None and ci < len(extra):
                    extra[ci]()
                elif ci in (2, 4, 6):
                    pump()

        def rope_norm(prj, g, out_ap):
            gsl = bass.ts(g, SG)
            sq = work.tile([128, SG], f32r, tag="sq2", name="sq2")
            nc.scalar.activation(sq[:], prj[:], AF.Square)
            psb = work.tile([128, SG], f32r, tag="psb", name="psb")
            nc.scalar.copy(psb[:], prj[:])
            t1 = work.tile([128, SG], f32, tag="t1", name="t1")
            nc.vector.tensor_mul(t1[:], prj[:], css[g][:])

            def tail1():
                ssq2 = ps_row.tile([2, SG], f32, tag="r1", name="ssq2")
                nc.tensor.matmul(ssq2[:], hm[:], sq[:], start=True, stop=True)
                swp = ps_sw.tile([128, SG], f32, tag="sw", name="swp_ps")
                nc.tensor.matmul(swp[:], psw[:], psb[:], start=True, stop=True)
                sr2 = rows.tile([2, SG], f32, tag="sr2", name="sr2")
                nc.scalar.activation(sr2[:], ssq2[:], AF.Sqrt,
                                     bias=EPS, scale=1.0 / HD)
                rr2 = rows.tile([2, SG], f32r, tag="rr2", name="rr2")
                with nc.allow_low_precision(reason="rms bcast rows"):
                    nc.vector.reciprocal(rr2[:], sr2[:])
                t2 = work.tile([128, SG], f32, tag="t2", name="t2")
                nc.vector.tensor_mul(t2[:], swp[:], sss[g][:])
                rq = work.tile([128, SG], f32, tag="rq", name="rq")
                nc.vector.tensor_add(rq[:], t1[:], t2[:])

                def tail2():
                    bch = ps_bc.tile([128, SG], f32, tag="bc", name="bch")
                    nc.tensor.matmul(bch[:], hs[:], rr2[:],
                                     start=True, stop=True)
                    nc.vector.tensor_mul(out_ap, rq[:], bch[:])
                pend.append(tail2)
            pend.append(tail1)

        # ================= projections, by token group =================
        for g in range(NG):
            gsl = bass.ts(g, SG)
            if g + 1 < NG:
                load_group(g + 1)
            if g == 1:
                nc.sync.dma_start(wo_t[:], woT[:])
            xg = [xqs[g][:, i, :] for i in range(8)]
            # rms stats of raw x (only V needs the 1/rms scale)
            sqts = []
            for i in range(8):
                sqt = work.tile([128, SG], f32r, tag=f"sqs{i % 2}",
                                name=f"sqs{i}", bufs=2)
                nc.gpsimd.tensor_mul(sqt[:], xg[i].bitcast(f32),
                                     xg[i].bitcast(f32))
                sqts.append(sqt)
            ssq = ps_row.tile([1, SG], f32, tag="r1", name="ssq")

            def mk_ssq(i):
                def f():
                    nc.tensor.matmul(ssq[:], ones_col[:], sqts[i][:],
                                     start=(i == 0), stop=(i == 7))
                return f

            # Q0: interleave the 8 group-rms reduce matmuls 1:1
            prj = ps_acc.tile([128, SG], f32, tag="acc", name="q_ps")
            proj_mms(prj, lambda ci: wq_t[:, ci, 0:128], xg,
                     extra=[mk_ssq(i) for i in range(8)])
            rope_norm(prj, g, qT[0][:, gsl])
            srow = rows.tile([1, SG], f32, tag="srow", name="srow")
            nc.scalar.activation(srow[:], ssq[:], AF.Sqrt,
                                 bias=EPS, scale=1.0 / C)
            rrow = rows.tile([1, SG], f32r, tag="rrow", name="rrow")
            with nc.allow_low_precision(reason="rms bcast"):
                nc.vector.reciprocal(rrow[:], srow[:])

            # Q1, K
            prj = ps_acc.tile([128, SG], f32, tag="acc", name="q_ps")
            proj_mms(prj, lambda ci: wq_t[:, ci, 128:256], xg)
            rope_norm(prj, g, qT[1][:, gsl])
            prj = ps_acc.tile([128, SG], f32, tag="acc", name="k_ps")
            proj_mms(prj, lambda ci: wk_t[:, ci, :], xg)
            rope_norm(prj, g, kT[:, gsl])

            # V: feature-major (scaled by 1/rms) then transpose to token-major
            vps = ps_acc.tile([128, SG], f32, tag="acc", name="v_ps")
            proj_mms(vps, lambda ci: wv_t[:, ci, :], xg)

            def mk_vtail(g, vps, rrow):
                def f():
                    bc = ps_bc.tile([128, SG], f32, tag="bc", name="bc")
                    nc.tensor.matmul(bc[:], ones_row[:], rrow[:],
                                     start=True, stop=True)
                    bcs = work.tile([128, SG], f32, tag="bcs", name="bcs")
                    nc.vector.tensor_copy(bcs[:], bc[:])
                    vsb = work.tile([128, SG], f32r, tag="vsb", name="vsb")
                    nc.vector.tensor_mul(vsb[:], vps[:], bcs[:])

                    def tr(tt0):
                        def h():
                            for tt in (tt0, tt0 + 1):
                                vtp = ps_sw.tile([128, 128], f32r, tag="sw",
                                                 name="vtp")
                                nc.tensor.matmul(vtp[:],
                                                 vsb[:, bass.ts(tt, 128)],
                                                 pid[:], is_transpose=True,
                                                 start=True, stop=True)
                                va3 = vaug[g * 4 + tt]
                                nc.vector.tensor_copy(
                                    va3[:, :, 0:HD],
                                    vtp[:].rearrange("p (h d) -> p h d", d=HD))
                        return h
                    pend.append(tr(0))
                    pend.append(tr(2))
                return f
            pend.append(mk_vtail(g, vps, rrow))
        while pend:
            pump()

        # ================= attention + wo, by query group =================
        # The normalize tail of head hp is emitted AFTER the first score
        # matmuls of head hp+1, so the in-order PE queue never stalls on the
        # (DVE) reciprocal at head boundaries.
        pending = None

        def emit_normalize():
            nonlocal pending
            if pending is None:
                return
            yv, yTd, qb = pending
            pending = None
            ry = rows.tile([1, SG], f32r, tag="ry", name="ry")
            with nc.allow_low_precision(reason="softmax denom"):
                nc.vector.reciprocal(ry[:], yv[HD:HD + 1, :])
            ybc = ps_bc.tile([64, SG], f32, tag="bc", name="ybc")
            nc.tensor.matmul(ybc[:], ones_row[:, 0:64], ry[:],
                             start=True, stop=True)
            ybcs = work.tile([128, SG], f32, tag="ybcs", name="ybcs")
            nc.vector.tensor_copy(ybcs[qb:qb + 64, :], ybc[:])
            nc.vector.tensor_mul(yTd[qb:qb + 64, :],
                                 yv[0:HD, :], ybcs[qb:qb + 64, :])

        # scores run AHEAD=2 slots before the matching yv so the in-order PE
        # queue never stalls on the Act-engine exp.
        AHEAD = 2
        for qg in range(4):
            qsl = bass.ts(qg, SG)
            yT = [ypool.tile([128, SG], f32r, tag=f"yT{d}", name=f"yT{d}")
                  for d in range(2)]
            for hp in range(4):
                dq, qb = hp % 2, 64 * (hp // 2)
                yv = ps_yv.tile([HD + 1, SG], f32, tag="yv", name="yv")
                nslots = 4 * qg + 4
                live = [s for s in range(nslots)
                        if (qg * SG + max(0, 128 * (s - 4 * qg))
                            - (128 * s + 127)) <= window]
                pTs = {}

                def emit_score(s):
                    off = max(0, 128 * (s - 4 * qg))
                    cols = SG - off
                    dmax = qg * SG + SG - 1 - 128 * s
                    sps = ps_acc.tile([128, SG], f32, tag="acc", name="sps")
                    nc.tensor.matmul(
                        sps[:, 0:cols],
                        kT[qb:qb + 64, bass.ts(s, 128)],
                        qT[dq][qb:qb + 64, qg * SG + off:(qg + 1) * SG],
                        start=True, stop=True)
                    pT = pexp.tile([128, SG], f32r, tag="pT", name="pT")
                    nc.scalar.activation(pT[:, 0:cols], sps[:, 0:cols],
                                         AF.Exp, scale=0.125)
                    if s >= 4 * qg:
                        nc.gpsimd.tensor_mul(pT[:, 0:128], pT[:, 0:128],
                                             dmask[:])
                    if dmax > window:
                        nc.gpsimd.affine_select(
                            pT[:, 0:cols], pT[:, 0:cols],
                            pattern=[[1, cols]],
                            base=qg * SG + off - 128 * s - window,
                            channel_multiplier=-1,
                            compare_op=mybir.AluOpType.is_le, fill=0.0)
                    pTs[s] = (pT, off, cols)

                for j in range(min(AHEAD, len(live))):
                    emit_score(live[j])
                for idx, s in enumerate(live):
                    if idx + AHEAD < len(live):
                        emit_score(live[idx + AHEAD])
                    pT, off, cols = pTs.pop(s)
                    nc.tensor.matmul(yv[:, off:SG], vaug[s][:, hp // 2, :],
                                     pT[:, 0:cols],
                                     start=(idx == 0), stop=(s == live[-1]))
                    if idx == 1 and pending is not None:
                        emit_normalize()
                if pending is not None:
                    emit_normalize()
                pending = (yv, yT[dq][:], qb)
            emit_normalize()
            # wo partial for this query group (stores batched 4 chunks/DMA)
            yof = y_out.rearrange("(a p) t -> p a t", p=128)
            for co in range(8):
                ops = ps_acc.tile([128, SG], f32, tag="acc", name="o_ps")
                for ci in range(2):
                    nc.tensor.matmul(ops[:], wo_t[:, ci, bass.ts(co, 128)],
                                     yT[ci][:], start=(ci == 0), stop=(ci == 1))
                if co % 4 == 0:
                    ob = obuf.tile([128, 4, SG], f32, tag="ob", name="ob")
                if co % 4 == 0:
                    nc.scalar.copy(ob[:, 0, :], ops[:])
                else:
                    nc.vector.tensor_copy(ob[:, co % 4, :], ops[:])
                if co % 4 == 3:
                    nc.sync.dma_start(yof[:, co - 3:co + 1, qsl], ob[:])

    nc.compile()
    return nc


# --------------------------------------------------------------------------
# Phase 2: expert MLP, fp8 DoubleRow with hi/lo split quantization
# --------------------------------------------------------------------------
def build_phase2(ncap: int):
    nc = bacc.Bacc("TRN2", target_bir_lowering=False, debug=False,
                   num_devices=NCORES)
    NT = ncap // 512

    xhi = nc.dram_tensor("xhi", [128, 8, ncap], fp8, kind="ExternalInput").ap()
    xlo = nc.dram_tensor("xlo", [128, 8, ncap], fp8, kind="ExternalInput").ap()
    wuph = nc.dram_tensor("wuph", [128, 8, HID], fp8, kind="ExternalInput").ap()
    wuplo = nc.dram_tensor("wuplo", [128, 8, HID], fp8, kind="ExternalInput").ap()
    wdnh = nc.dram_tensor("wdnh", [128, 16, C], fp8, kind="ExternalInput").ap()
    wdnlo = nc.dram_tensor("wdnlo", [128, 16, C], fp8, kind="ExternalInput").ap()
    moe_out = nc.dram_tensor("moe_out", [C, ncap], bf16, kind="ExternalOutput").ap()

    from contextlib import ExitStack
    with tile.TileContext(nc) as tc, ExitStack() as est:
        wpool = est.enter_context(tc.tile_pool(name="wpool", bufs=1))
        hpool = est.enter_context(tc.tile_pool(name="hpool", bufs=2))
        stream = est.enter_context(tc.tile_pool(name="stream", bufs=1))
        work = est.enter_context(tc.tile_pool(name="work", bufs=3))
        ps_h = est.enter_context(tc.tile_pool(name="ps_h", bufs=4, space="PSUM"))
        ps_o = est.enter_context(tc.tile_pool(name="ps_o", bufs=4, space="PSUM"))

        xh_t = stream.tile([128, 8, ncap], fp8, name="xh_t")
        nc.scalar.dma_start(xh_t[:], xhi[:])
        xl_t = stream.tile([128, 8, ncap], fp8, name="xl_t")
        nc.scalar.dma_start(xl_t[:], xlo[:])

        uph_t = wpool.tile([128, 8, HID], fp8, name="uph_t")
        uplo_t = wpool.tile([128, 8, HID], fp8, name="uplo_t")
        dnh_t = wpool.tile([128, 16, C], fp8, name="dnh_t")
        dnlo_t = wpool.tile([128, 16, C], fp8, name="dnlo_t")
        for j in range(2):
            js = bass.ts(j, HID // 2)
            nc.sync.dma_start(uph_t[:, :, js], wuph[:, :, js])
            nc.sync.dma_start(uplo_t[:, :, js], wuplo[:, :, js])
        nc.sync.dma_start(dnh_t[:], wdnh[:])
        nc.sync.dma_start(dnlo_t[:], wdnlo[:])

        for nt in range(NT):
            csl = bass.ts(nt, 512)
            hT = hpool.tile([128, 16, 512], fp8, tag="hT", name="hT")
            for hc in range(16):
                h_ps = ps_h.tile([128, 512], f32, tag="h", name="h_ps")
                hsl = bass.ts(hc, 128)
                for c2 in range(4):
                    p2 = slice(2 * c2, 2 * c2 + 2)
                    nc.tensor.matmul(h_ps[:], uph_t[:, p2, hsl],
                                     xh_t[:, p2, csl], start=(c2 == 0),
                                     stop=False, perf_mode=DR)
                for c2 in range(4):
                    p2 = slice(2 * c2, 2 * c2 + 2)
                    nc.tensor.matmul(h_ps[:], uph_t[:, p2, hsl],
                                     xl_t[:, p2, csl], start=False,
                                     stop=False, perf_mode=DR)
                for c2 in range(4):
                    p2 = slice(2 * c2, 2 * c2 + 2)
                    nc.tensor.matmul(h_ps[:], uplo_t[:, p2, hsl],
                                     xh_t[:, p2, csl], start=False,
                                     stop=(c2 == 3), perf_mode=DR)
                hr = work.tile([128, 512], f32, tag="hr", name="hr")
                nc.scalar.activation(hr[:], h_ps[:], AF.Relu, scale=1.0 / WU)
                with nc.allow_low_precision(reason="fp8 hidden"):
                    nc.vector.tensor_mul(hT[:, hc, :], hr[:], hr[:])
            for co in range(8):
                o_ps = ps_o.tile([128, 512], f32, tag="o", name="o_ps")
                osl = bass.ts(co, 128)
                for i2 in range(8):
                    p2 = slice(2 * i2, 2 * i2 + 2)
                    nc.tensor.matmul(o_ps[:], dnh_t[:, p2, osl],
                                     hT[:, p2, :], start=(i2 == 0),
                                     stop=False, perf_mode=DR)
                for i2 in range(8):
                    p2 = slice(2 * i2, 2 * i2 + 2)
                    nc.tensor.matmul(o_ps[:], dnlo_t[:, p2, osl],
                                     hT[:, p2, :], start=False,
                                     stop=(i2 == 7), perf_mode=DR)
                if co % 4 == 0:
                    ot = work.tile([128, 4, 512], bf16, tag="ot", name="ot")
                with nc.allow_low_precision(reason="bf16 out"):
                    if co % 2 == 0:
                        nc.scalar.copy(ot[:, co % 4, :], o_ps[:])
                    else:
                        nc.vector.tensor_copy(ot[:, co % 4, :], o_ps[:])
                if co % 4 == 3:
                    mof = moe_out.rearrange("(a p) n -> p a n", p=128)
                    nc.sync.dma_start(mof[:, co - 3:co + 1, csl], ot[:])

    nc.compile()
    return nc


# --------------------------------------------------------------------------
# Host orchestration
# --------------------------------------------------------------------------
def _q_cols(g):
    cols = []
    for h_ in (0, 2, 1, 3):
        h = 4 * g + h_
        cols += list(range(h * HD, (h + 1) * HD))
    return np.array(cols, np.int64)


def _phase1_inputs(x, cos, sin, wq, wk, wv, wo):
    cosT = np.ascontiguousarray(cos[0, :, 0, :].T)  # (32, T)
    sinT = np.ascontiguousarray(sin[0, :, 0, :].T)
    cosR = np.tile(cosT, (4, 1)).astype(np.float32)
    sinR = np.tile(np.vstack([sinT, -sinT]), (2, 1)).astype(np.float32)

    psw = np.zeros((128, 128), np.float32)
    psw[np.arange(128) ^ 32, np.arange(128)] = 1.0
    pident = np.eye(128, dtype=np.float32)
    hmask = np.zeros((128, 2), np.float32)
    hmask[0:64, 0] = 1.0
    hmask[64:128, 1] = 1.0
    hsel = np.ascontiguousarray(hmask.T)

    xTb = [_fold(x[b].T, 8) for b in range(B)]
    in_maps = []
    for c in range(NCORES):
        b, g = c // 4, c % 4
        qc = _q_cols(g)
        kvs = slice(128 * g, 128 * g + 128)
        in_maps.append(dict(
            xT=xTb[b], cosR=cosR, sinR=sinR,
            wqT=_fold(wq.T[:, qc], 8),
            wkT=_fold(wk.T[:, kvs], 8),
            wvT=_fold(wv.T[:, kvs], 8),
            woT=_fold(wo.T[qc, :], 2),
            pswap=psw, pident=pident, hmask=hmask, hsel=hsel,
        ))
    return in_maps


def _route(logits, router_bias):
    sig = (1.0 / (1.0 + np.exp(-logits.astype(np.float32)))).astype(np.float32)
    sel = sig + router_bias[None, :].astype(np.float32)
    idx = np.argsort(-sel, axis=1, kind="stable")[:, :TOPK]
    tw = np.take_along_axis(sig, idx, axis=1)
    tw = tw / tw.sum(axis=1, keepdims=True)
    N = logits.shape[0]
    sparse_w = np.zeros((N, E_MLP + E_VE), np.float32)
    np.put_along_axis(sparse_w, idx, tw, axis=1)
    return sparse_w


def _fold(a, nchunk):
    a = np.asarray(a)
    return np.ascontiguousarray(
        a.reshape(nchunk, 128, a.shape[-1]).transpose(1, 0, 2))


def kernel(**inputs):
    x = np.asarray(inputs["x"], np.float32)
    token_ids = np.asarray(inputs["token_ids"])
    cos = np.asarray(inputs["cos"], np.float32)
    sin = np.asarray(inputs["sin"], np.float32)
    window = int(np.asarray(inputs["window_size"]))
    wq, wk, wv, wo = (np.asarray(inputs[k], np.float32)
                      for k in ("wq", "wk", "wv", "wo"))
    w_up = np.asarray(inputs["w_up"], np.float32)
    w_down = np.asarray(inputs["w_down"], np.float32)
    router_w = np.asarray(inputs["router_w"], np.float32)
    router_bias = np.asarray(inputs["router_bias"], np.float32)
    ve_tables = np.asarray(inputs["ve_tables"], np.float32)

    key1 = ("p1", window)
    if key1 not in _prog_cache:
        _prog_cache[key1] = build_phase1(window)
    nc1 = _prog_cache[key1]

    in_maps = _phase1_inputs(x, cos, sin, wq, wk, wv, wo)
    res1 = run_bass_kernel_spmd(nc1, in_maps, list(range(NCORES))).results

    # host: reduce partials, residual, rmsnorm, router
    x2 = np.empty((B, T, C), np.float32)
    for b in range(B):
        acc = x[b].T.copy()
        for g in range(4):
            acc += res1[4 * b + g]["y_out"]
        x2[b] = acc.T
    x2f = x2.reshape(-1, C)
    xf = x2f / np.sqrt((x2f * x2f).mean(1, keepdims=True) + EPS)
    xf = xf.astype(np.float32)
    logits = xf @ router_w.T
    sparse_w = _route(logits, router_bias)

    # dispatch
    ncap = NCAP
    idx_list, n_list = [], []
    for e in range(E_MLP):
        idx_e = np.nonzero(sparse_w[:, e])[0]
        idx_list.append(idx_e)
        n_list.append(len(idx_e))
    max_n = max(n_list)
    while ncap < max_n:
        ncap *= 2

    key2 = ("p2", ncap)
    if key2 not in _prog_cache:
        _prog_cache[key2] = build_phase2(ncap)
    nc2 = _prog_cache[key2]

    xfT = np.ascontiguousarray(xf.T)  # (C, N)
    in_maps2 = []
    for e in range(NCORES):
        idx_e = idx_list[e]
        n_e = n_list[e]
        xe = np.zeros((C, ncap), np.float32)
        xe[:, :n_e] = xfT[:, idx_e]
        xh = xe.astype(E4NP)
        xl = (xe - xh.astype(np.float32)).astype(E4NP)
        wu = w_up[e].T * WU          # (C, HID)
        wuh = wu.astype(E4NP)
        wul = (wu - wuh.astype(np.float32)).astype(E4NP)
        wd = w_down[e].T * WD        # (HID, C)
        wdh = wd.astype(E4NP)
        wdl = (wd - wdh.astype(np.float32)).astype(E4NP)
        in_maps2.append(dict(
            xhi=_fold(xh, 8), xlo=_fold(xl, 8),
            wuph=_fold(wuh, 8), wuplo=_fold(wul, 8),
            wdnh=_fold(wdh, 16), wdnlo=_fold(wdl, 16)))
    res2 = run_bass_kernel_spmd(nc2, in_maps2, list(range(NCORES))).results

    out = x2f.copy()
    # VE experts on host (pure gather + scale)
    tok = token_ids.reshape(-1)
    for ee in range(E_VE):
        w = sparse_w[:, E_MLP + ee]
        nz = np.nonzero(w)[0]
        out[nz] += w[nz, None] * ve_tables[ee][tok[nz]]
    for e in range(E_MLP):
        n_e = n_list[e]
        if n_e:
            moe = res2[e]["moe_out"][:, :n_e]
            g = (sparse_w[idx_list[e], e] / WD).astype(np.float32)
            out[idx_list[e]] += g[:, None] * moe.astype(np.float32).T
    return out.reshape(B, T, C).astype(np.float32)


# revision 6
# speedup vs baseline: 1.1932x; 1.0457x over previous
"""TRN2 Bass kernel v2 for nn_BlockMoVaE — head-sharded attention.

Sharding:
  Phase 1 (attention): core c = (b, g) with b = c//4, g = c%4 computes heads
    {4g..4g+3} (kv heads {2g, 2g+1}) of batch b over ALL T=2048 tokens, and
    outputs the wo-projected partial y contribution [C, T].  Each core sees
    the full causal triangle, so no slot is dead and total score/AV work is
    halved vs token-sharding.  Host reduces the 4 partials per batch, adds
    the residual, computes rmsnorm + router logits (tiny O(N*C) work) and
    does top-2 routing exactly as the reference.
  Phase 2 (expert MLP): core e runs expert e over its routed tokens
    (capacity-padded).  fp8(e4m3) weights/activations with DoubleRow
    matmuls; both weight matrices and the up-proj input are SPLIT into
    hi + lo fp8 pairs (3-pass up, 2-pass down) so the only surviving
    quantization error is the fp8 hidden state (~1e-2 of output max).
    VE (vocab-embedding) experts are a pure host-side gather + scale.

Key identity: rmsnorm(rope(rmsnorm(x) @ wq)) == rmsnorm(rope(x @ wq)) per
head, because the per-token rms scale commutes through the projection and
rope and cancels in the final per-head rmsnorm.  So Q/K are projected from
RAW x; only V needs the per-token 1/rms scale (applied on the psum copy).
"""
import numpy as np
import ml_dtypes

import concourse.bass as bass
import concourse.bacc as bacc
import concourse.mybir as mybir
import concourse.tile as tile
from concourse.bass_utils import run_bass_kernel_spmd

# ---- problem constants ----
B, T, C = 2, 2048, 1024
NH, NKV, HD = 16, 8, 64
E_MLP, E_VE, TOPK = 8, 2, 2
HID = 2048
VOCAB = 50257
EPS = 1e-6
NCORES = 8
NSLOT = T // 128      # 16
NG = 4                # token groups of 512 in phase 1
SG = T // NG          # 512
NCAP = 1024           # phase-2 expert capacity
WU = 64.0             # fp8 up-weight pre-scale
WD = 64.0             # fp8 down-weight pre-scale

f32 = mybir.dt.float32
f32r = mybir.dt.float32r
bf16 = mybir.dt.bfloat16
fp8 = mybir.dt.float8e4
E4NP = ml_dtypes.float8_e4m3
BF16NP = ml_dtypes.bfloat16
AF = mybir.ActivationFunctionType
DR = mybir.MatmulPerfMode.DoubleRow

_prog_cache = {}


def _register_consts(nc, values):
    for value in values:
        key = (f32, float(value))
        if key not in nc.const_aps.aps:
            t = nc.alloc_sbuf_tensor(f"constap-{value}", [128, 1], f32)
            nc.gpsimd.memset(t.ap(), float(value))
            nc.const_aps.aps[key] = t.ap()
    nc.all_engine_barrier()


# --------------------------------------------------------------------------
# Phase 1: head-sharded attention, partial wo output
# --------------------------------------------------------------------------
def build_phase1(window: int):
    nc = bacc.Bacc("TRN2", target_bir_lowering=False, debug=False,
                   num_devices=NCORES)

    xT = nc.dram_tensor("xT", [128, 8, T], f32r, kind="ExternalInput").ap()
    cosR = nc.dram_tensor("cosR", [128, T], f32, kind="ExternalInput").ap()
    sinR = nc.dram_tensor("sinR", [128, T], f32, kind="ExternalInput").ap()
    wqT = nc.dram_tensor("wqT", [128, 8, 256], f32r, kind="ExternalInput").ap()
    wkT = nc.dram_tensor("wkT", [128, 8, 128], f32r, kind="ExternalInput").ap()
    wvT = nc.dram_tensor("wvT", [128, 8, 128], f32r, kind="ExternalInput").ap()
    woT = nc.dram_tensor("woT", [128, 2, C], f32r, kind="ExternalInput").ap()
    pswap = nc.dram_tensor("pswap", [128, 128], f32r, kind="ExternalInput").ap()
    pident = nc.dram_tensor("pident", [128, 128], f32r, kind="ExternalInput").ap()
    hmask = nc.dram_tensor("hmask", [128, 2], f32r, kind="ExternalInput").ap()
    hsel = nc.dram_tensor("hsel", [2, 128], f32r, kind="ExternalInput").ap()

    y_out = nc.dram_tensor("y_out", [C, T], f32, kind="ExternalOutput").ap()

    _register_consts(nc, [EPS, 0.0])
    from contextlib import ExitStack
    with tile.TileContext(nc) as tc, ExitStack() as est:
        const = est.enter_context(tc.tile_pool(name="const", bufs=1))
        wpool = est.enter_context(tc.tile_pool(name="wpool", bufs=1))
        ropes = est.enter_context(tc.tile_pool(name="ropes", bufs=2))
        xin = est.enter_context(tc.tile_pool(name="xin", bufs=2))
        qkv = est.enter_context(tc.tile_pool(name="qkv", bufs=1))
        vap = est.enter_context(tc.tile_pool(name="vap", bufs=1))
        work = est.enter_context(tc.tile_pool(name="work", bufs=2))
        rows = est.enter_context(tc.tile_pool(name="rows", bufs=2))
        pexp = est.enter_context(tc.tile_pool(name="pexp", bufs=4))
        ypool = est.enter_context(tc.tile_pool(name="ypool", bufs=2))
        obuf = est.enter_context(tc.tile_pool(name="obuf", bufs=3))
        # PSUM banks: acc(3) + sw(1) + row(1) + bc(1) + yv(2) = 8
        ps_acc = est.enter_context(tc.tile_pool(name="ps_acc", bufs=3, space="PSUM"))
        ps_sw = est.enter_context(tc.tile_pool(name="ps_sw", bufs=1, space="PSUM"))
        ps_row = est.enter_context(tc.tile_pool(name="ps_row", bufs=1, space="PSUM"))
        ps_bc = est.enter_context(tc.tile_pool(name="ps_bc", bufs=1, space="PSUM"))
        ps_yv = est.enter_context(tc.tile_pool(name="ps_yv", bufs=2, space="PSUM"))

        # ---- constants ----
        ones_col_f = const.tile([128, 1], f32, name="ones_col_f")
        nc.vector.memset(ones_col_f[:], 1.0)
        ones_col = const.tile([128, 1], f32r, name="ones_col")
        nc.scalar.copy(ones_col[:], ones_col_f[:])
        ones_row_f = const.tile([1, 128], f32, name="ones_row_f")
        nc.vector.memset(ones_row_f[:], 1.0)
        ones_row = const.tile([1, 128], f32r, name="ones_row")
        nc.scalar.copy(ones_row[:], ones_row_f[:])
        # x group 0 + rope-table slices go FIRST on the (single) SP DMA
        # queue so group-0 compute starts immediately; weights follow.
        xqs = [None] * NG
        css = [None] * NG
        sss = [None] * NG

        def load_group(g):
            xqs[g] = xin.tile([128, 8, SG], f32r, tag="x", name="xq")
            nc.sync.dma_start(xqs[g][:], xT[:, :, bass.ts(g, SG)])
            css[g] = ropes.tile([128, SG], f32, tag="cs", name="cs")
            nc.sync.dma_start(css[g][:], cosR[:, bass.ts(g, SG)])
            sss[g] = ropes.tile([128, SG], f32, tag="ss", name="ss")
            nc.sync.dma_start(sss[g][:], sinR[:, bass.ts(g, SG)])

        # group-0 startup: first half of x, then Q weights, then the rest,
        # so the first projection matmuls can begin ~4us in.
        wq_t = wpool.tile([128, 8, 256], f32r, name="wq_t")
        wk_t = wpool.tile([128, 8, 128], f32r, name="wk_t")
        wv_t = wpool.tile([128, 8, 128], f32r, name="wv_t")
        wo_t = wpool.tile([128, 2, C], f32r, name="wo_t")
        xqs[0] = xin.tile([128, 8, SG], f32r, tag="x", name="xq")
        nc.sync.dma_start(xqs[0][:, 0:4, :], xT[:, 0:4, 0:SG])
        nc.sync.dma_start(wq_t[:], wqT[:])
        nc.sync.dma_start(xqs[0][:, 4:8, :], xT[:, 4:8, 0:SG])
        nc.sync.dma_start(wk_t[:], wkT[:])
        nc.sync.dma_start(wv_t[:], wvT[:])
        css[0] = ropes.tile([128, SG], f32, tag="cs", name="cs")
        nc.sync.dma_start(css[0][:], cosR[:, 0:SG])
        sss[0] = ropes.tile([128, SG], f32, tag="ss", name="ss")
        nc.sync.dma_start(sss[0][:], sinR[:, 0:SG])
        psw = const.tile([128, 128], f32r, name="psw")
        nc.sync.dma_start(psw[:], pswap[:])
        hm = const.tile([128, 2], f32r, name="hm")
        nc.sync.dma_start(hm[:], hmask[:])
        hs = const.tile([2, 128], f32r, name="hs")
        nc.sync.dma_start(hs[:], hsel[:])
        pid = const.tile([128, 128], f32r, name="pid")
        nc.sync.dma_start(pid[:], pident[:])

        qT = [qkv.tile([128, T], f32r, name=f"qT{d}") for d in range(2)]
        kT = qkv.tile([128, T], f32r, name="kT")
        vaug = [vap.tile([128, 2, HD + 1], f32r, name=f"va{s}")
                for s in range(NSLOT)]
        onescols = const.tile([128, 2, 1], f32, name="onescols")
        nc.vector.memset(onescols[:], 1.0)
        for s in range(NSLOT):
            nc.gpsimd.tensor_copy(vaug[s][:, :, HD:HD + 1], onescols[:])
        dmask = const.tile([128, 128], f32, name="dmask")
        nc.gpsimd.memset(dmask[:], 1.0)
        nc.gpsimd.affine_select(dmask[:], dmask[:], pattern=[[1, 128]],
                                base=0, channel_multiplier=-1,
                                compare_op=mybir.AluOpType.is_ge, fill=0.0)

        # ---- software-pipelined projections ----
        # Each chunk's cross-engine tail (swap matmul, rms reduce, rope
        # vector chain) is deferred into the NEXT chunk's projection-matmul
        # stream via a FIFO of closures, so the in-order PE queue never
        # stalls waiting for Act/DVE results.
        pend = []

        def pump():
            if pend:
                pend.pop(0)()

        def proj_mms(prj, lhsT_of, xg, extra=None):
            for ci in range(8):
                nc.tensor.matmul(prj[:], lhsT_of(ci), xg[ci],
                                 start=(ci == 0), stop=(ci == 7))
                if extra is not None and ci < len(extra):
                    extra[ci]()
                elif ci in (2, 4, 6):
                    pump()

        def rope_norm(prj, g, out_ap):
            gsl = bass.ts(g, SG)
            sq = work.tile([128, SG], f32r, tag="sq2", name="sq2")
            nc.scalar.activation(sq[:], prj[:], AF.Square)
            psb = work.tile([128, SG], f32r, tag="psb", name="psb")
            nc.scalar.copy(psb[:], prj[:])
            t1 = work.tile([128, SG], f32, tag="t1", name="t1")
            nc.vector.tensor_mul(t1[:], prj[:], css[g][:])

            def tail1():
                ssq2 = ps_row.tile([2, SG], f32, tag="r1", name="ssq2")
                nc.tensor.matmul(ssq2[:], hm[:], sq[:], start=True, stop=True)
                swp = ps_sw.tile([128, SG], f32, tag="sw", name="swp_ps")
                nc.tensor.matmul(swp[:], psw[:], psb[:], start=True, stop=True)
                sr2 = rows.tile([2, SG], f32, tag="sr2", name="sr2")
                nc.scalar.activation(sr2[:], ssq2[:], AF.Sqrt,
                                     bias=EPS, scale=1.0 / HD)
                rr2 = rows.tile([2, SG], f32r, tag="rr2", name="rr2")
                with nc.allow_low_precision(reason="rms bcast rows"):
                    nc.vector.reciprocal(rr2[:], sr2[:])
                t2 = work.tile([128, SG], f32, tag="t2", name="t2")
                nc.vector.tensor_mul(t2[:], swp[:], sss[g][:])
                rq = work.tile([128, SG], f32, tag="rq", name="rq")
                nc.vector.tensor_add(rq[:], t1[:], t2[:])

                def tail2():
                    bch = ps_bc.tile([128, SG], f32, tag="bc", name="bch")
                    nc.tensor.matmul(bch[:], hs[:], rr2[:],
                                     start=True, stop=True)
                    nc.vector.tensor_mul(out_ap, rq[:], bch[:])
                pend.append(tail2)
            pend.append(tail1)

        # ================= projections, by token group =================
        for g in range(NG):
            gsl = bass.ts(g, SG)
            if g + 1 < NG:
                load_group(g + 1)
            if g == 1:
                nc.sync.dma_start(wo_t[:], woT[:])
            xg = [xqs[g][:, i, :] for i in range(8)]
            # rms stats of raw x (only V needs the 1/rms scale)
            sqts = []
            for i in range(8):
                sqt = work.tile([128, SG], f32r, tag=f"sqs{i % 2}",
                                name=f"sqs{i}", bufs=2)
                nc.gpsimd.tensor_mul(sqt[:], xg[i].bitcast(f32),
                                     xg[i].bitcast(f32))
                sqts.append(sqt)
            ssq = ps_row.tile([1, SG], f32, tag="r1", name="ssq")

            def mk_ssq(i):
                def f():
                    nc.tensor.matmul(ssq[:], ones_col[:], sqts[i][:],
                                     start=(i == 0), stop=(i == 7))
                return f

            # Q0: interleave the 8 group-rms reduce matmuls 1:1
            prj = ps_acc.tile([128, SG], f32, tag="acc", name="q_ps")
            proj_mms(prj, lambda ci: wq_t[:, ci, 0:128], xg,
                     extra=[mk_ssq(i) for i in range(8)])
            rope_norm(prj, g, qT[0][:, gsl])
            srow = rows.tile([1, SG], f32, tag="srow", name="srow")
            nc.scalar.activation(srow[:], ssq[:], AF.Sqrt,
                                 bias=EPS, scale=1.0 / C)
            rrow = rows.tile([1, SG], f32r, tag="rrow", name="rrow")
            with nc.allow_low_precision(reason="rms bcast"):
                nc.vector.reciprocal(rrow[:], srow[:])

            # Q1, K
            prj = ps_acc.tile([128, SG], f32, tag="acc", name="q_ps")
            proj_mms(prj, lambda ci: wq_t[:, ci, 128:256], xg)
            rope_norm(prj, g, qT[1][:, gsl])
            prj = ps_acc.tile([128, SG], f32, tag="acc", name="k_ps")
            proj_mms(prj, lambda ci: wk_t[:, ci, :], xg)
            rope_norm(prj, g, kT[:, gsl])

            # V: feature-major (scaled by 1/rms) then transpose to token-major
            vps = ps_acc.tile([128, SG], f32, tag="acc", name="v_ps")
            proj_mms(vps, lambda ci: wv_t[:, ci, :], xg)

            def mk_vtail(g, vps, rrow):
                def f():
                    bc = ps_bc.tile([128, SG], f32, tag="bc", name="bc")
                    nc.tensor.matmul(bc[:], ones_row[:], rrow[:],
                                     start=True, stop=True)
                    bcs = work.tile([128, SG], f32, tag="bcs", name="bcs")
                    nc.vector.tensor_copy(bcs[:], bc[:])
                    vsb = work.tile([128, SG], f32r, tag="vsb", name="vsb")
                    nc.vector.tensor_mul(vsb[:], vps[:], bcs[:])

                    def tr(tt0):
                        def h():
                            for tt in (tt0, tt0 + 1):
                                vtp = ps_sw.tile([128, 128], f32r, tag="sw",
                                                 name="vtp")
                                nc.tensor.matmul(vtp[:],
                                                 vsb[:, bass.ts(tt, 128)],
                                                 pid[:], is_transpose=True,
                                                 start=True, stop=True)
                                va3 = vaug[g * 4 + tt]
                                nc.vector.tensor_copy(
                                    va3[:, :, 0:HD],
                                    vtp[:].rearrange("p (h d) -> p h d", d=HD))
                        return h
                    pend.append(tr(0))
                    pend.append(tr(2))
                return f
            pend.append(mk_vtail(g, vps, rrow))
        while pend:
            pump()

        # ================= attention + wo, by query group =================
        # The normalize tail of head hp is emitted AFTER the first score
        # matmuls of head hp+1, so the in-order PE queue never stalls on the
        # (DVE) reciprocal at head boundaries.
        pending = None

        def emit_normalize():
            nonlocal pending
            if pending is None:
                return
            yv, yTd, qb = pending
            pending = None
            ry = rows.tile([1, SG], f32r, tag="ry", name="ry")
            with nc.allow_low_precision(reason="softmax denom"):
                nc.vector.reciprocal(ry[:], yv[HD:HD + 1, :])
            ybc = ps_bc.tile([64, SG], f32, tag="bc", name="ybc")
            nc.tensor.matmul(ybc[:], ones_row[:, 0:64], ry[:],
                             start=True, stop=True)
            ybcs = work.tile([128, SG], f32, tag="ybcs", name="ybcs")
            nc.vector.tensor_copy(ybcs[qb:qb + 64, :], ybc[:])
            nc.vector.tensor_mul(yTd[qb:qb + 64, :],
                                 yv[0:HD, :], ybcs[qb:qb + 64, :])

        # scores run AHEAD=2 slots before the matching yv so the in-order PE
        # queue never stalls on the Act-engine exp.
        AHEAD = 2
        for qg in range(4):
            qsl = bass.ts(qg, SG)
            yT = [ypool.tile([128, SG], f32r, tag=f"yT{d}", name=f"yT{d}")
                  for d in range(2)]
            for hp in range(4):
                dq, qb = hp % 2, 64 * (hp // 2)
                yv = ps_yv.tile([HD + 1, SG], f32, tag="yv", name="yv")
                nslots = 4 * qg + 4
                live = [s for s in range(nslots)
                        if (qg * SG + max(0, 128 * (s - 4 * qg))
                            - (128 * s + 127)) <= window]
                pTs = {}

                def emit_score(s):
                    off = max(0, 128 * (s - 4 * qg))
                    cols = SG - off
                    dmax = qg * SG + SG - 1 - 128 * s
                    sps = ps_acc.tile([128, SG], f32, tag="acc", name="sps")
                    nc.tensor.matmul(
                        sps[:, 0:cols],
                        kT[qb:qb + 64, bass.ts(s, 128)],
                        qT[dq][qb:qb + 64, qg * SG + off:(qg + 1) * SG],
                        start=True, stop=True)
                    pT = pexp.tile([128, SG], f32r, tag="pT", name="pT")
                    nc.scalar.activation(pT[:, 0:cols], sps[:, 0:cols],
                                         AF.Exp, scale=0.125)
                    if s >= 4 * qg:
                        nc.gpsimd.tensor_mul(pT[:, 0:128], pT[:, 0:128],
                                             dmask[:])
                    if dmax > window:
                        nc.gpsimd.affine_select(
                            pT[:, 0:cols], pT[:, 0:cols],
                            pattern=[[1, cols]],
                            base=qg * SG + off - 128 * s - window,
                            channel_multiplier=-1,
                            compare_op=mybir.AluOpType.is_le, fill=0.0)
                    pTs[s] = (pT, off, cols)

                for j in range(min(AHEAD, len(live))):
                    emit_score(live[j])
                for idx, s in enumerate(live):
                    if idx + AHEAD < len(live):
                        emit_score(live[idx + AHEAD])
                    pT, off, cols = pTs.pop(s)
                    nc.tensor.matmul(yv[:, off:SG], vaug[s][:, hp // 2, :],
                                     pT[:, 0:cols],
                                     start=(idx == 0), stop=(s == live[-1]))
                    if idx == 1 and pending is not None:
                        emit_normalize()
                if pending is not None:
                    emit_normalize()
                pending = (yv, yT[dq][:], qb)
            emit_normalize()
            # wo partial for this query group (stores batched 4 chunks/DMA)
            yof = y_out.rearrange("(a p) t -> p a t", p=128)
            for co in range(8):
                ops = ps_acc.tile([128, SG], f32, tag="acc", name="o_ps")
                for ci in range(2):
                    nc.tensor.matmul(ops[:], wo_t[:, ci, bass.ts(co, 128)],
                                     yT[ci][:], start=(ci == 0), stop=(ci == 1))
                if co % 4 == 0:
                    ob = obuf.tile([128, 4, SG], f32, tag="ob", name="ob")
                if co % 4 == 0:
                    nc.scalar.copy(ob[:, 0, :], ops[:])
                else:
                    nc.vector.tensor_copy(ob[:, co % 4, :], ops[:])
                if co % 4 == 3:
                    nc.sync.dma_start(yof[:, co - 3:co + 1, qsl], ob[:])

    nc.compile()
    return nc


# --------------------------------------------------------------------------
# Phase 2: expert MLP, fp8 DoubleRow with hi/lo split quantization
# --------------------------------------------------------------------------
def build_phase2(ncap: int):
    nc = bacc.Bacc("TRN2", target_bir_lowering=False, debug=False,
                   num_devices=NCORES)
    NT = ncap // 512

    xhi = nc.dram_tensor("xhi", [128, 8, ncap], fp8, kind="ExternalInput").ap()
    xlo = nc.dram_tensor("xlo", [128, 8, ncap], fp8, kind="ExternalInput").ap()
    wuph = nc.dram_tensor("wuph", [128, 8, HID], fp8, kind="ExternalInput").ap()
    wuplo = nc.dram_tensor("wuplo", [128, 8, HID], fp8, kind="ExternalInput").ap()
    wdnh = nc.dram_tensor("wdnh", [128, 16, C], fp8, kind="ExternalInput").ap()
    wdnlo = nc.dram_tensor("wdnlo", [128, 16, C], fp8, kind="ExternalInput").ap()
    moe_out = nc.dram_tensor("moe_out", [C, ncap], bf16, kind="ExternalOutput").ap()

    from contextlib import ExitStack
    with tile.TileContext(nc) as tc, ExitStack() as est:
        wpool = est.enter_context(tc.tile_pool(name="wpool", bufs=1))
        hpool = est.enter_context(tc.tile_pool(name="hpool", bufs=2))
        stream = est.enter_context(tc.tile_pool(name="stream", bufs=1))
        work = est.enter_context(tc.tile_pool(name="work", bufs=3))
        ps_h = est.enter_context(tc.tile_pool(name="ps_h", bufs=4, space="PSUM"))
        ps_o = est.enter_context(tc.tile_pool(name="ps_o", bufs=4, space="PSUM"))

        xh_t = stream.tile([128, 8, ncap], fp8, name="xh_t")
        nc.scalar.dma_start(xh_t[:], xhi[:])
        xl_t = stream.tile([128, 8, ncap], fp8, name="xl_t")
        nc.scalar.dma_start(xl_t[:], xlo[:])

        uph_t = wpool.tile([128, 8, HID], fp8, name="uph_t")
        uplo_t = wpool.tile([128, 8, HID], fp8, name="uplo_t")
        dnh_t = wpool.tile([128, 16, C], fp8, name="dnh_t")
        dnlo_t = wpool.tile([128, 16, C], fp8, name="dnlo_t")
        for j in range(4):
            js = bass.ts(j, HID // 4)
            nc.sync.dma_start(uph_t[:, :, js], wuph[:, :, js])
            nc.sync.dma_start(uplo_t[:, :, js], wuplo[:, :, js])
        nc.sync.dma_start(dnh_t[:], wdnh[:])
        nc.sync.dma_start(dnlo_t[:], wdnlo[:])

        for nt in range(NT):
            csl = bass.ts(nt, 512)
            hT = hpool.tile([128, 16, 512], fp8, tag="hT", name="hT")
            for hc in range(16):
                h_ps = ps_h.tile([128, 512], f32, tag="h", name="h_ps")
                hsl = bass.ts(hc, 128)
                for c2 in range(4):
                    p2 = slice(2 * c2, 2 * c2 + 2)
                    nc.tensor.matmul(h_ps[:], uph_t[:, p2, hsl],
                                     xh_t[:, p2, csl], start=(c2 == 0),
                                     stop=False, perf_mode=DR)
                for c2 in range(4):
                    p2 = slice(2 * c2, 2 * c2 + 2)
                    nc.tensor.matmul(h_ps[:], uph_t[:, p2, hsl],
                                     xl_t[:, p2, csl], start=False,
                                     stop=False, perf_mode=DR)
                for c2 in range(4):
                    p2 = slice(2 * c2, 2 * c2 + 2)
                    nc.tensor.matmul(h_ps[:], uplo_t[:, p2, hsl],
                                     xh_t[:, p2, csl], start=False,
                                     stop=(c2 == 3), perf_mode=DR)
                hr = work.tile([128, 512], f32, tag="hr", name="hr")
                nc.scalar.activation(hr[:], h_ps[:], AF.Relu, scale=1.0 / WU)
                with nc.allow_low_precision(reason="fp8 hidden"):
                    nc.vector.tensor_mul(hT[:, hc, :], hr[:], hr[:])
            for co in range(8):
                o_ps = ps_o.tile([128, 512], f32, tag="o", name="o_ps")
                osl = bass.ts(co, 128)
                for i2 in range(8):
                    p2 = slice(2 * i2, 2 * i2 + 2)
                    nc.tensor.matmul(o_ps[:], dnh_t[:, p2, osl],
                                     hT[:, p2, :], start=(i2 == 0),
                                     stop=False, perf_mode=DR)
                for i2 in range(8):
                    p2 = slice(2 * i2, 2 * i2 + 2)
                    nc.tensor.matmul(o_ps[:], dnlo_t[:, p2, osl],
                                     hT[:, p2, :], start=False,
                                     stop=(i2 == 7), perf_mode=DR)
                if co % 4 == 0:
                    ot = work.tile([128, 4, 512], bf16, tag="ot", name="ot")
                with nc.allow_low_precision(reason="bf16 out"):
                    if co % 2 == 0:
                        nc.scalar.copy(ot[:, co % 4, :], o_ps[:])
                    else:
                        nc.vector.tensor_copy(ot[:, co % 4, :], o_ps[:])
                if co % 4 == 3:
                    mof = moe_out.rearrange("(a p) n -> p a n", p=128)
                    nc.sync.dma_start(mof[:, co - 3:co + 1, csl], ot[:])

    nc.compile()
    return nc


# --------------------------------------------------------------------------
# Host orchestration
# --------------------------------------------------------------------------
def _q_cols(g):
    cols = []
    for h_ in (0, 2, 1, 3):
        h = 4 * g + h_
        cols += list(range(h * HD, (h + 1) * HD))
    return np.array(cols, np.int64)


def _phase1_inputs(x, cos, sin, wq, wk, wv, wo):
    cosT = np.ascontiguousarray(cos[0, :, 0, :].T)  # (32, T)
    sinT = np.ascontiguousarray(sin[0, :, 0, :].T)
    cosR = np.tile(cosT, (4, 1)).astype(np.float32)
    sinR = np.tile(np.vstack([sinT, -sinT]), (2, 1)).astype(np.float32)

    psw = np.zeros((128, 128), np.float32)
    psw[np.arange(128) ^ 32, np.arange(128)] = 1.0
    pident = np.eye(128, dtype=np.float32)
    hmask = np.zeros((128, 2), np.float32)
    hmask[0:64, 0] = 1.0
    hmask[64:128, 1] = 1.0
    hsel = np.ascontiguousarray(hmask.T)

    xTb = [_fold(x[b].T, 8) for b in range(B)]
    in_maps = []
    for c in range(NCORES):
        b, g = c // 4, c % 4
        qc = _q_cols(g)
        kvs = slice(128 * g, 128 * g + 128)
        in_maps.append(dict(
            xT=xTb[b], cosR=cosR, sinR=sinR,
            wqT=_fold(wq.T[:, qc], 8),
            wkT=_fold(wk.T[:, kvs], 8),
            wvT=_fold(wv.T[:, kvs], 8),
            woT=_fold(wo.T[qc, :], 2),
            pswap=psw, pident=pident, hmask=hmask, hsel=hsel,
        ))
    return in_maps


def _route(logits, router_bias):
    sig = (1.0 / (1.0 + np.exp(-logits.astype(np.float32)))).astype(np.float32)
    sel = sig + router_bias[None, :].astype(np.float32)
    idx = np.argsort(-sel, axis=1, kind="stable")[:, :TOPK]
    tw = np.take_along_axis(sig, idx, axis=1)
    tw = tw / tw.sum(axis=1, keepdims=True)
    N = logits.shape[0]
    sparse_w = np.zeros((N, E_MLP + E_VE), np.float32)
    np.put_along_axis(sparse_w, idx, tw, axis=1)
    return sparse_w


def _fold(a, nchunk):
    a = np.asarray(a)
    return np.ascontiguousarray(
        a.reshape(nchunk, 128, a.shape[-1]).transpose(1, 0, 2))


def kernel(**inputs):
    x = np.asarray(inputs["x"], np.float32)
    token_ids = np.asarray(inputs["token_ids"])
    cos = np.asarray(inputs["cos"], np.float32)
    sin = np.asarray(inputs["sin"], np.float32)
    window = int(np.asarray(inputs["window_size"]))
    wq, wk, wv, wo = (np.asarray(inputs[k], np.float32)
                      for k in ("wq", "wk", "wv", "wo"))
    w_up = np.asarray(inputs["w_up"], np.float32)
    w_down = np.asarray(inputs["w_down"], np.float32)
    router_w = np.asarray(inputs["router_w"], np.float32)
    router_bias = np.asarray(inputs["router_bias"], np.float32)
    ve_tables = np.asarray(inputs["ve_tables"], np.float32)

    key1 = ("p1", window)
    if key1 not in _prog_cache:
        _prog_cache[key1] = build_phase1(window)
    nc1 = _prog_cache[key1]

    in_maps = _phase1_inputs(x, cos, sin, wq, wk, wv, wo)
    res1 = run_bass_kernel_spmd(nc1, in_maps, list(range(NCORES))).results

    # host: reduce partials, residual, rmsnorm, router
    x2 = np.empty((B, T, C), np.float32)
    for b in range(B):
        acc = x[b].T.copy()
        for g in range(4):
            acc += res1[4 * b + g]["y_out"]
        x2[b] = acc.T
    x2f = x2.reshape(-1, C)
    xf = x2f / np.sqrt((x2f * x2f).mean(1, keepdims=True) + EPS)
    xf = xf.astype(np.float32)
    logits = xf @ router_w.T
    sparse_w = _route(logits, router_bias)

    # dispatch
    ncap = NCAP
    idx_list, n_list = [], []
    for e in range(E_MLP):
        idx_e = np.nonzero(sparse_w[:, e])[0]
        idx_list.append(idx_e)
        n_list.append(len(idx_e))
    max_n = max(n_list)
    while ncap < max_n:
        ncap *= 2

    key2 = ("p2", ncap)
    if key2 not in _prog_cache:
        _prog_cache[key2] = build_phase2(ncap)
    nc2 = _prog_cache[key2]

    xfT = np.ascontiguousarray(xf.T)  # (C, N)
    in_maps2 = []
    for e in range(NCORES):
        idx_e = idx_list[e]
        n_e = n_list[e]
        xe = np.zeros((C, ncap), np.float32)
        xe[:, :n_e] = xfT[:, idx_e]
        xh = xe.astype(E4NP)
        xl = (xe - xh.astype(np.float32)).astype(E4NP)
        wu = w_up[e].T * WU          # (C, HID)
        wuh = wu.astype(E4NP)
        wul = (wu - wuh.astype(np.float32)).astype(E4NP)
        wd = w_down[e].T * WD        # (HID, C)
        wdh = wd.astype(E4NP)
        wdl = (wd - wdh.astype(np.float32)).astype(E4NP)
        in_maps2.append(dict(
            xhi=_fold(xh, 8), xlo=_fold(xl, 8),
            wuph=_fold(wuh, 8), wuplo=_fold(wul, 8),
            wdnh=_fold(wdh, 16), wdnlo=_fold(wdl, 16)))
    res2 = run_bass_kernel_spmd(nc2, in_maps2, list(range(NCORES))).results

    out = x2f.copy()
    # VE experts on host (pure gather + scale)
    tok = token_ids.reshape(-1)
    for ee in range(E_VE):
        w = sparse_w[:, E_MLP + ee]
        nz = np.nonzero(w)[0]
        out[nz] += w[nz, None] * ve_tables[ee][tok[nz]]
    for e in range(E_MLP):
        n_e = n_list[e]
        if n_e:
            moe = res2[e]["moe_out"][:, :n_e]
            g = (sparse_w[idx_list[e], e] / WD).astype(np.float32)
            out[idx_list[e]] += g[:, None] * moe.astype(np.float32).T
    return out.reshape(B, T, C).astype(np.float32)
